# revision 21
# baseline (speedup 1.0000x reference)
"""Deformable cross-attention on 8 trn2 NeuronCores via Bass/Tile.

Sharding: core c owns head c (both batch elements); host sums the 8
per-head partials of the output projection.

Per core: offset MLP (Woff1@Wq folded on host), per-head per-offset
projected kv maps P via PE matmuls (bf16), one-descriptor-per-sample
bilinear gather from a quad-row P4 layout via dma_gather, flash-style
attention over key tiles with multiplicative exp(bias) windows, and the
per-head slice of the output projection.
"""

import numpy as np
import ml_dtypes

bf16 = ml_dtypes.bfloat16

B, DIM, H, W = 2, 256, 56, 56
HEADS, O, OS = 8, 9, 0.1
HD = DIM // HEADS          # 32
HW = H * W                 # 3136
SCALE = np.float32(HD ** -0.5)
PT = 25                    # position tiles of 128
NP = PT * 128              # 3200 padded positions
IC = 448                   # attention i-chunk (8 rows of y_i)
NCH = HW // IC             # 7
EBT = 111 * 111

_STATE: dict = {}


def _host_prep(inputs):
    f32 = np.float32
    Wq = np.asarray(inputs['Wq'], f32)
    Wk = np.asarray(inputs['Wk'], f32)
    Wv = np.asarray(inputs['Wv'], f32)
    Woff1 = np.asarray(inputs['Woff1'], f32)
    Woff2 = np.asarray(inputs['Woff2'], f32)
    Wout = np.asarray(inputs['Wout'], f32)
    wfoldT = np.ascontiguousarray((Woff1 @ Wq).T).astype(bf16)       # (256,64)
    woff2T = np.ascontiguousarray(Woff2.T).astype(bf16)              # (64,18)
    boff1c = np.asarray(inputs['boff1'], f32).reshape(64, 1)
    boff2c = np.asarray(inputs['boff2'], f32).reshape(18, 1)

    dyv = np.arange(-(H - 1), H, dtype=f32) / (H - 1)
    dxv = np.arange(-(W - 1), W, dtype=f32) / (W - 1)
    gy, gx = np.meshgrid(dyv, dxv, indexing='ij')
    coords = np.stack([gy, gx], -1).reshape(-1, 2).astype(f32)
    hdn = np.maximum(coords @ np.asarray(inputs['cpb_w1'], f32).T
                     + np.asarray(inputs['cpb_b1'], f32), 0.0)
    table = hdn @ np.asarray(inputs['cpb_w2'], f32).T + np.asarray(inputs['cpb_b2'], f32)
    ebt = np.exp(table.astype(f32))                                  # (12321, 8)
    # expand over x_j so EB window DMAs have all-positive steps:
    # ebx[h, x_j, a, x_i] = exp(T2)[a, x_i + 55 - x_j, h]
    eb2 = ebt.reshape(111, 111, HEADS)
    xj = np.arange(W)[:, None, None]
    aa = np.arange(111)[None, :, None]
    xi = np.arange(W)[None, None, :]
    ebx = eb2[aa, xi + (W - 1) - xj, :].astype(bf16)      # (56, 111, 56, 8)

    pos = np.arange(NP)
    py_pos = np.where(pos < HW, pos // W, 0)
    px_pos = np.where(pos < HW, pos % W, 0)
    gxl = np.linspace(-1.0, 1.0, W, dtype=f32)
    gyl = np.linspace(-1.0, 1.0, H, dtype=f32)
    basex = ((gxl[px_pos] + 1.0) * 0.5 * (W - 1)).astype(f32)
    basey = ((gyl[py_pos] + 1.0) * 0.5 * (H - 1)).astype(f32)
    lane = pos % 128
    tcol = pos // 128
    xgc2 = np.zeros((128, O * PT), f32)
    ygc2 = np.zeros((128, O * PT), f32)
    obase2 = np.zeros((128, O * PT), f32)
    for o in range(O):
        xgc2[lane, o * PT + tcol] = basex
        ygc2[lane, o * PT + tcol] = basey
        obase2[lane, o * PT + tcol] = o

    x32 = np.ascontiguousarray(np.asarray(inputs['query_map'], f32).reshape(B, DIM, HW))
    kv32 = np.ascontiguousarray(np.asarray(inputs['kv_map'], f32).reshape(B, DIM, HW))
    per_core = []
    for h in range(HEADS):
        wpack = np.zeros((DIM, O * 64), f32)
        for o in range(O):
            wpack[:, o * 64:o * 64 + 32] = Wk[h * HD:(h + 1) * HD, o * DIM:(o + 1) * DIM].T
            wpack[:, o * 64 + 32:o * 64 + 64] = Wv[h * HD:(h + 1) * HD, o * DIM:(o + 1) * DIM].T
        per_core.append({
            'x32': x32, 'kv32': kv32,
            'wfoldT': wfoldT, 'boff1c': boff1c,
            'woff2T': woff2T, 'boff2c': boff2c,
            'wqhT': np.ascontiguousarray(Wq[h * HD:(h + 1) * HD].T).astype(bf16),
            'wpack': wpack.astype(bf16),
            'wouthT': np.ascontiguousarray(Wout[:, h * HD:(h + 1) * HD].T).astype(bf16),
            'ebtab': np.ascontiguousarray(ebx[..., h]).reshape(1, W * 111 * W),
            'xgc2': xgc2, 'ygc2': ygc2, 'obase2': obase2,
        })
    return per_core


def _build_program():
    import os
    STAGE = int(os.environ.get('KSTAGE', '4'))
    SUB = int(os.environ.get('KSUB', '9'))
    import concourse.bass as bass
    import concourse.bacc as baccm
    import concourse.tile as tile
    from concourse import mybir
    from concourse.masks import make_identity
    import contextlib

    f32 = mybir.dt.float32
    b16d = mybir.dt.bfloat16
    i16 = mybir.dt.int16
    ALU = mybir.AluOpType
    ACTF = mybir.ActivationFunctionType

    nc = baccm.Bacc(trn_type="TRN2")

    x32 = nc.dram_tensor('x32', [B, DIM, HW], f32, kind='ExternalInput')
    kv32 = nc.dram_tensor('kv32', [B, DIM, HW], f32, kind='ExternalInput')
    wfoldT = nc.dram_tensor('wfoldT', [DIM, 64], b16d, kind='ExternalInput')
    boff1c = nc.dram_tensor('boff1c', [64, 1], f32, kind='ExternalInput')
    woff2T = nc.dram_tensor('woff2T', [64, 18], b16d, kind='ExternalInput')
    boff2c = nc.dram_tensor('boff2c', [18, 1], f32, kind='ExternalInput')
    wqhT = nc.dram_tensor('wqhT', [DIM, HD], b16d, kind='ExternalInput')
    wpack = nc.dram_tensor('wpack', [DIM, O * 64], b16d, kind='ExternalInput')
    wouthT = nc.dram_tensor('wouthT', [HD, DIM], b16d, kind='ExternalInput')
    ebtab = nc.dram_tensor('ebtab', [1, W * 111 * W], b16d, kind='ExternalInput')
    xgc2 = nc.dram_tensor('xgc2', [128, O * PT], f32, kind='ExternalInput')
    ygc2 = nc.dram_tensor('ygc2', [128, O * PT], f32, kind='ExternalInput')
    obase2 = nc.dram_tensor('obase2', [128, O * PT], f32, kind='ExternalInput')
    outp = nc.dram_tensor('outp', [B, DIM, HW], b16d, kind='ExternalOutput')

    offd = nc.dram_tensor('offd', [B, 18, HW], f32, kind='Internal')
    idxd = nc.dram_tensor('idxd', [B, O * NP], i16, kind='Internal')
    p4d = nc.dram_tensor('p4d', [B, NP * O * 256], b16d, kind='Internal')

    def dap(handle, off, dims):
        return bass.AP(tensor=handle, offset=off, ap=[list(d) for d in dims])

    with tile.TileContext(nc) as tc:
        ctx = contextlib.ExitStack()
        consts = ctx.enter_context(tc.tile_pool(name='consts', bufs=1))
        big = ctx.enter_context(tc.tile_pool(name='big', bufs=1))
        work = ctx.enter_context(tc.tile_pool(name='work', bufs=2))
        gpool = ctx.enter_context(tc.tile_pool(name='gpool', bufs=1))
        ntt = ctx.enter_context(tc.tile_pool(name='ntt', bufs=3))
        st_ps = ctx.enter_context(tc.tile_pool(name='st_ps', bufs=2, space='PSUM'))
        av_ps = ctx.enter_context(tc.tile_pool(name='av_ps', bufs=2, space='PSUM'))
        misc_ps = ctx.enter_context(tc.tile_pool(name='misc_ps', bufs=2, space='PSUM'))

        with ctx:
            wfoldT_s = consts.tile([128, 2, 64], b16d)
            nc.sync.dma_start(out=wfoldT_s, in_=wfoldT.rearrange('(c l) k -> l c k', c=2))
            boff1_s = consts.tile([64, 1], f32)
            nc.sync.dma_start(out=boff1_s, in_=boff1c[:, :])
            woff2T_s = consts.tile([64, 18], b16d)
            nc.sync.dma_start(out=woff2T_s, in_=woff2T[:, :])
            boff2_s = consts.tile([18, 1], f32)
            nc.sync.dma_start(out=boff2_s, in_=boff2c[:, :])
            wqhT_s = consts.tile([128, 2, HD], b16d)
            nc.sync.dma_start(out=wqhT_s, in_=wqhT.rearrange('(c l) k -> l c k', c=2))
            wpack_s = consts.tile([128, 2, O * 64], b16d)
            nc.sync.dma_start(out=wpack_s, in_=wpack.rearrange('(c l) k -> l c k', c=2))
            wouthT_s = consts.tile([HD, DIM], b16d)
            nc.sync.dma_start(out=wouthT_s, in_=wouthT[:, :])
            xgc2_s = consts.tile([128, O * PT], f32)
            nc.sync.dma_start(out=xgc2_s, in_=xgc2[:, :])
            ygc2_s = consts.tile([128, O * PT], f32)
            nc.sync.dma_start(out=ygc2_s, in_=ygc2[:, :])
            obase2_s = consts.tile([128, O * PT], f32)
            nc.sync.dma_start(out=obase2_s, in_=obase2[:, :])
            ident = consts.tile([128, 128], b16d)
            make_identity(nc, ident)
            ones32 = consts.tile([1, HD], f32)
            nc.vector.memset(ones32, 1.0)

            kvt_all = [big.tile([128, PT, 65], b16d, tag=f'kvt{b}', name=f'kvt{b}') for b in range(B)]
            k4_all = [big.tile([HD, PT * 128], b16d, tag=f'k4{b}', name=f'k4{b}') for b in range(B)]
            qs_all = [big.tile([HD, HW], b16d, tag=f'qs{b}', name=f'qs{b}') for b in range(B)]
            for b in range(B):
                nc.vector.memset(kvt_all[b][:, :, 64:65], 1.0)

            for b in range(B):
                x_bf = big.tile([128, 2, HW], b16d, tag='x_bf')
                kv_bf = big.tile([128, 2, HW], b16d, tag='kv_bf')
                for cc in range(2):
                    nc.gpsimd.dma_start(out=x_bf[:, cc, :],
                                        in_=x32[b, cc * 128:(cc + 1) * 128, :])
                    nc.gpsimd.dma_start(out=kv_bf[:, cc, :],
                                        in_=kv32[b, cc * 128:(cc + 1) * 128, :])

                # ---- offset MLP ----
                off_s = big.tile([18, HW], f32, tag='off')
                for ch in range(NCH):
                    sl = slice(ch * IC, (ch + 1) * IC)
                    hid_ps = misc_ps.tile([64, IC], f32, tag='mps')
                    for cc in range(2):
                        nc.tensor.matmul(hid_ps, wfoldT_s[:, cc, :], x_bf[:, cc, sl],
                                         start=(cc == 0), stop=(cc == 1))
                    hid_s = work.tile([64, IC], b16d, tag='hid')
                    nc.scalar.activation(hid_s, hid_ps, ACTF.Gelu_apprx_tanh, bias=boff1_s)
                    off_ps = misc_ps.tile([18, IC], f32, tag='mps')
                    nc.tensor.matmul(off_ps, woff2T_s, hid_s, start=True, stop=True)
                    nc.scalar.activation(off_s[:, sl], off_ps, ACTF.Identity, bias=boff2_s)
                nc.sync.dma_start(out=offd[b, :, :], in_=off_s)
                offx = work.tile([128, O * PT], f32, tag='offx')
                offy = work.tile([128, O * PT], f32, tag='offy')
                nc.vector.memset(offx, 0.0)
                nc.vector.memset(offy, 0.0)
                for xy, dst in ((0, offx), (1, offy)):
                    for o in range(O):
                        base = (b * 18 + xy * O + o) * HW
                        nc.sync.dma_start(
                            out=dst[:, o * PT:o * PT + 24],
                            in_=dap(offd, base, [[1, 128], [128, 24]]))
                        nc.sync.dma_start(
                            out=dst[:64, o * PT + 24:o * PT + 25],
                            in_=dap(offd, base + 24 * 128, [[1, 64], [128, 1]]))

                # ---- coords / weights / indices ----
                px = work.tile([128, O * PT], f32, tag='px')
                py = work.tile([128, O * PT], f32, tag='py')
                nc.vector.tensor_scalar(px, offx, float(OS * 0.5 * (W - 1)), None, ALU.mult)
                nc.vector.tensor_tensor(px, px, xgc2_s, ALU.add)
                nc.vector.tensor_scalar(px, px, 0.0, float(W - 1), ALU.max, ALU.min)
                nc.vector.tensor_scalar(py, offy, float(OS * 0.5 * (H - 1)), None, ALU.mult)
                nc.vector.tensor_tensor(py, py, ygc2_s, ALU.add)
                nc.vector.tensor_scalar(py, py, 0.0, float(H - 1), ALU.max, ALU.min)
                MAGIC = 12582912.0  # 1.5 * 2**23: float32 round-to-int trick
                x0f = work.tile([128, O * PT], f32, tag='x0f')
                y0f = work.tile([128, O * PT], f32, tag='y0f')
                nc.vector.tensor_scalar(x0f, px, -0.5, MAGIC, ALU.add, ALU.add)
                nc.vector.tensor_scalar(x0f, x0f, -MAGIC, None, ALU.add)
                nc.vector.tensor_scalar(y0f, py, -0.5, MAGIC, ALU.add, ALU.add)
                nc.vector.tensor_scalar(y0f, y0f, -MAGIC, None, ALU.add)
                wx = work.tile([128, O * PT], f32, tag='wx')
                wy = work.tile([128, O * PT], f32, tag='wy')
                nc.vector.tensor_tensor(wx, px, x0f, ALU.subtract)
                nc.vector.tensor_tensor(wy, py, y0f, ALU.subtract)
                r0 = work.tile([128, O * PT], f32, tag='r0')
                nc.vector.tensor_scalar(r0, y0f, float(W), None, ALU.mult)
                nc.vector.tensor_tensor(r0, r0, x0f, ALU.add)
                nc.vector.tensor_scalar(r0, r0, float(O), None, ALU.mult)
                nc.vector.tensor_tensor(r0, r0, obase2_s, ALU.add)
                nc.vector.tensor_scalar(r0, r0, 0.0, float(NP * O - 1), ALU.max, ALU.min)
                idx16 = work.tile([128, O * PT], i16, tag='idx')
                nc.vector.tensor_copy(idx16, r0)
                nc.sync.dma_start(
                    out=dap(idxd, b * O * NP, [[1, 128], [NP, O], [128, PT]]),
                    in_=idx16)
                idxw = big.tile([128, O * (NP // 16)], i16, tag='idxw')
                for rep in range(8):
                    nc.sync.dma_start(
                        out=idxw[rep * 16:(rep + 1) * 16, :],
                        in_=dap(idxd, b * O * NP,
                                [[1, 16], [NP, O], [16, NP // 16]]))
                w4 = big.tile([128, O * PT, 4], b16d, tag='w4')
                onemwx = work.tile([128, O * PT], f32, tag='o1x')
                onemwy = work.tile([128, O * PT], f32, tag='o1y')
                nc.vector.tensor_scalar(onemwx, wx, -1.0, 1.0, ALU.mult, ALU.add)
                nc.vector.tensor_scalar(onemwy, wy, -1.0, 1.0, ALU.mult, ALU.add)
                wtmp = work.tile([128, O * PT], f32, tag='wtmp')
                for n, (aa, cc2) in enumerate(((onemwx, onemwy), (wx, onemwy),
                                               (onemwx, wy), (wx, wy))):
                    nc.vector.tensor_tensor(wtmp, aa, cc2, ALU.mult)
                    nc.vector.tensor_copy(w4[:, :, n], wtmp)

                # ---- P projection ----
                if STAGE < 2:
                    continue
                p_sb = big.tile([128, 26, O * 64], b16d, tag='p_sb')
                nc.vector.memset(p_sb[:, 24:26, :], 0.0)
                for it in range(PT):
                    rows = 128 if it < 24 else HW - 24 * 128
                    for n2 in range(2):
                        pps = misc_ps.tile([128, 288], f32, tag='mps')
                        for cc in range(2):
                            nc.tensor.matmul(
                                pps[:rows],
                                kv_bf[:, cc, it * 128:it * 128 + rows],
                                wpack_s[:, cc, n2 * 288:(n2 + 1) * 288],
                                start=(cc == 0), stop=(cc == 1))
                        nc.scalar.activation(p_sb[:rows, it, n2 * 288:(n2 + 1) * 288],
                                             pps[:rows], ACTF.Copy)
                # ---- P4 build ----
                for n, dlt in enumerate((0, 1, 56, 57)):
                    lo = dlt
                    first = 128 - lo
                    for o in range(O):
                        base = b * NP * O * 256 + o * 256 + n * 64
                        nc.sync.dma_start(
                            out=dap(p4d, base,
                                    [[O * 256, first], [128 * O * 256, PT], [1, 64]]),
                            in_=p_sb[lo:128, 0:PT, o * 64:(o + 1) * 64])
                        if lo > 0:
                            nc.sync.dma_start(
                                out=dap(p4d, base + first * O * 256,
                                        [[O * 256, lo], [128 * O * 256, PT], [1, 64]]),
                                in_=p_sb[0:lo, 1:PT + 1, o * 64:(o + 1) * 64])

                # ---- gather + bilinear ----
                if STAGE < 3:
                    continue
                for o in range(O):
                    gt = gpool.tile([128, PT, 256], b16d, tag='gt', bufs=2)
                    nc.gpsimd.dma_gather(
                        out_ap=gt,
                        in_ap=dap(p4d, b * NP * O * 256, [[256, NP * O], [1, 256]]),
                        idxs_ap=idxw[:, o * (NP // 16):(o + 1) * (NP // 16)],
                        num_idxs=NP, num_idxs_reg=NP,
                        elem_size=256, elem_step=256, single_packet=False)
                    if SUB < 1:
                        continue
                    w4x = gpool.tile([128, PT, 4, 64], b16d, tag='w4x')
                    w4sl = w4[:, o * PT:(o + 1) * PT, :]
                    nc.gpsimd.tensor_copy(
                        w4x,
                        bass.AP(tensor=w4sl.tensor, offset=w4sl.offset,
                                ap=[*w4sl.ap, [0, 64]]))
                    if SUB < 2:
                        continue
                    nc.vector.tensor_tensor(
                        gt.rearrange('l t c -> l (t c)'),
                        gt.rearrange('l t c -> l (t c)'),
                        w4x.rearrange('l t n c -> l (t n c)'), ALU.mult)
                    gt4 = gt.rearrange('l t (n c) -> l t n c', c=64)
                    t12 = gpool.tile([128, PT, 2, 64], b16d, tag='t12')
                    nc.vector.tensor_tensor(t12, gt4[:, :, 0:2, :], gt4[:, :, 2:4, :],
                                            ALU.add)
                    if o == 0:
                        nc.vector.tensor_tensor(kvt_all[b][:, :, 0:64],
                                                t12[:, :, 0, :], t12[:, :, 1, :],
                                                ALU.add)
                    else:
                        nc.vector.tensor_tensor(kvt_all[b][:, :, 0:64],
                                                kvt_all[b][:, :, 0:64], t12[:, :, 0, :],
                                                ALU.add)
                        nc.vector.tensor_tensor(kvt_all[b][:, :, 0:64],
                                                kvt_all[b][:, :, 0:64], t12[:, :, 1, :],
                                                ALU.add)

                # ---- k transposes ----
                for t in (range(PT) if SUB >= 3 else []):
                    tp_ps = misc_ps.tile([HD, 128], b16d, tag='mps')
                    nc.tensor.transpose(tp_ps, kvt_all[b][:, t, 0:HD], ident)
                    nc.scalar.activation(k4_all[b][:, t * 128:(t + 1) * 128], tp_ps,
                                         ACTF.Copy)

                # ---- q_h scaled ----
                for ch in (range(NCH) if SUB >= 4 else []):
                    sl = slice(ch * IC, (ch + 1) * IC)
                    q_ps = misc_ps.tile([HD, IC], f32, tag='mps')
                    for cc in range(2):
                        nc.tensor.matmul(q_ps, wqhT_s[:, cc, :], x_bf[:, cc, sl],
                                         start=(cc == 0), stop=(cc == 1))
                    nc.scalar.activation(qs_all[b][:, sl], q_ps, ACTF.Copy,
                                         scale=float(SCALE))

            # ---------- attention ----------
            if STAGE < 4:
                for b in range(B):
                    dummy = work.tile([128, HW], b16d, tag='dmy')
                    nc.vector.memset(dummy, 0.25)
                    for cc in range(2):
                        nc.sync.dma_start(out=outp[b, cc * 128:(cc + 1) * 128, :],
                                          in_=dummy)
            jgroups = [(j, min(j + 2, PT)) for j in range(0, PT, 2)]
            for ic in (range(NCH) if STAGE >= 4 else []):
                yi0 = ic * 8
                avs = [av_ps.tile([33, IC], f32, tag=f'av{b}', bufs=1,
                                  name=f'av_{ic}_{b}') for b in range(B)]
                for (ja, jb) in jgroups:
                    jn = jb - ja
                    eb = ntt.tile([128, 2, IC], b16d, tag='eb')
                    for r in range(jn):
                        jt = ja + r
                        j0 = jt * 128
                        jend = min(j0 + 128, HW)
                        seg = j0
                        while seg < jend:
                            y_j = seg // W
                            seg_end = min((y_j + 1) * W, jend)
                            cnt = seg_end - seg
                            x_j0 = seg - y_j * W
                            basee = (x_j0 * 111 * W
                                     + (yi0 - y_j + (H - 1)) * W)
                            nc.sync.dma_start(
                                out=eb[seg - j0:seg - j0 + cnt, r, :]
                                    .rearrange('l (a c) -> l a c', a=8),
                                in_=dap(ebtab, basee,
                                        [[111 * W, cnt], [W, 8], [1, W]]))
                            seg = seg_end
                        if jend < j0 + 128:
                            nc.vector.memset(eb[jend - j0:128, r, :], 0.0)
                    for b in range(B):
                        stp = st_ps.tile([128, 2, 512], f32, tag='stp')
                        for r in range(jn):
                            jt = ja + r
                            nc.tensor.matmul(
                                stp[:, r, 0:IC],
                                k4_all[b][:, jt * 128:(jt + 1) * 128],
                                qs_all[b][:, ic * IC:(ic + 1) * IC],
                                start=True, stop=True)
                        nt = ntt.tile([128, 2, 512], b16d, tag='nt')
                        nc.scalar.activation(
                            nt[:, 0:jn, :].rearrange('l a c -> l (a c)'),
                            stp[:, 0:jn, :].rearrange('l a c -> l (a c)'),
                            ACTF.Exp)
                        nc.vector.tensor_tensor(
                            nt[:, 0:jn, 0:IC], nt[:, 0:jn, 0:IC],
                            eb[:, 0:jn, :], ALU.mult)
                        for r in range(jn):
                            jt = ja + r
                            nc.tensor.matmul(
                                avs[b], kvt_all[b][:, jt, 32:65], nt[:, r, 0:IC],
                                start=(jt == 0), stop=(jt == PT - 1),
                                skip_group_check=True)
                for b in range(B):
                    o1 = work.tile([33, IC], f32, tag='o1')
                    nc.scalar.activation(o1, avs[b], ACTF.Copy)
                    rc = work.tile([1, IC], f32, tag='rc')
                    nc.vector.reciprocal(rc, o1[32:33, :])
                    rb_ps = misc_ps.tile([HD, IC], f32, tag='mps')
                    nc.tensor.matmul(rb_ps, ones32, rc, start=True, stop=True)
                    att = work.tile([HD, IC], b16d, tag='att')
                    nc.vector.tensor_tensor(att, o1[0:HD, :], rb_ps, ALU.mult)
                    for mc in range(2):
                        wo_ps = misc_ps.tile([128, IC], f32, tag='mps')
                        nc.tensor.matmul(wo_ps, wouthT_s[:, mc * 128:(mc + 1) * 128],
                                         att, start=True, stop=True)
                        osb = work.tile([128, IC], b16d, tag='osb')
                        nc.scalar.activation(osb, wo_ps, ACTF.Copy)
                        nc.sync.dma_start(
                            out=outp[b, mc * 128:(mc + 1) * 128, ic * IC:(ic + 1) * IC],
                            in_=osb)
    nc.finalize()
    return nc


def _get_state():
    if 'nc' not in _STATE:
        _STATE['nc'] = _build_program()
    return _STATE['nc']


def kernel(query_map, kv_map, Wq, Wk, Wv, Woff1, boff1, Woff2, boff2,
           cpb_w1, cpb_b1, cpb_w2, cpb_b2, Wout, bout):
    from concourse.bass_utils import run_bass_kernel_spmd
    inputs = dict(query_map=query_map, kv_map=kv_map, Wq=Wq, Wk=Wk, Wv=Wv,
                  Woff1=Woff1, boff1=boff1, Woff2=Woff2, boff2=boff2,
                  cpb_w1=cpb_w1, cpb_b1=cpb_b1, cpb_w2=cpb_w2, cpb_b2=cpb_b2,
                  Wout=Wout, bout=bout)
    nc = _get_state()
    in_maps = _host_prep(inputs)
    res = run_bass_kernel_spmd(nc, in_maps, list(range(HEADS)))
    out = np.zeros((B, DIM, HW), np.float32)
    for c in range(HEADS):
        out += np.asarray(res.results[c]['outp']).astype(np.float32)
    out += np.asarray(bout, np.float32)[None, :, None]
    return out.reshape(B, DIM, H, W)


# revision 22
# speedup vs baseline: 1.3371x; 1.3371x over previous
"""Deformable cross-attention on 8 trn2 NeuronCores via Bass/Tile.

Sharding: core c owns head c (both batch elements); host sums the 8
per-head partials of the output projection.

Per core: offset MLP (Woff1@Wq folded on host), per-head per-offset
projected kv maps P via PE matmuls (bf16), one-descriptor-per-sample
bilinear gather from a quad-row P4 layout via dma_gather, flash-style
attention over key tiles with multiplicative exp(bias) windows, and the
per-head slice of the output projection.
"""

import numpy as np
import ml_dtypes

bf16 = ml_dtypes.bfloat16

B, DIM, H, W = 2, 256, 56, 56
HEADS, O, OS = 8, 9, 0.1
HD = DIM // HEADS          # 32
HW = H * W                 # 3136
SCALE = np.float32(HD ** -0.5)
PT = 25                    # position tiles of 128
NP = PT * 128              # 3200 padded positions
IC = 448                   # attention i-chunk (8 rows of y_i)
NCH = HW // IC             # 7
EBT = 111 * 111

_STATE: dict = {}


def _host_prep(inputs):
    f32 = np.float32
    Wq = np.asarray(inputs['Wq'], f32)
    Wk = np.asarray(inputs['Wk'], f32)
    Wv = np.asarray(inputs['Wv'], f32)
    Woff1 = np.asarray(inputs['Woff1'], f32)
    Woff2 = np.asarray(inputs['Woff2'], f32)
    Wout = np.asarray(inputs['Wout'], f32)
    wfoldT = np.ascontiguousarray((Woff1 @ Wq).T).astype(bf16)       # (256,64)
    woff2T = np.ascontiguousarray(Woff2.T).astype(bf16)              # (64,18)
    boff1c = np.asarray(inputs['boff1'], f32).reshape(64, 1)
    boff2c = np.asarray(inputs['boff2'], f32).reshape(18, 1)

    dyv = np.arange(-(H - 1), H, dtype=f32) / (H - 1)
    dxv = np.arange(-(W - 1), W, dtype=f32) / (W - 1)
    gy, gx = np.meshgrid(dyv, dxv, indexing='ij')
    coords = np.stack([gy, gx], -1).reshape(-1, 2).astype(f32)
    hdn = np.maximum(coords @ np.asarray(inputs['cpb_w1'], f32).T
                     + np.asarray(inputs['cpb_b1'], f32), 0.0)
    table = hdn @ np.asarray(inputs['cpb_w2'], f32).T + np.asarray(inputs['cpb_b2'], f32)
    ebt = np.exp(table.astype(f32))                                  # (12321, 8)
    # expand over x_j so EB window DMAs have all-positive steps:
    # ebx[h, x_j, a, x_i] = exp(T2)[a, x_i + 55 - x_j, h]
    eb2 = ebt.reshape(111, 111, HEADS)
    xj = np.arange(W)[:, None, None]
    aa = np.arange(111)[None, :, None]
    xi = np.arange(W)[None, None, :]
    ebx = eb2[aa, xi + (W - 1) - xj, :].astype(bf16)      # (56, 111, 56, 8)

    pos = np.arange(NP)
    py_pos = np.where(pos < HW, pos // W, 0)
    px_pos = np.where(pos < HW, pos % W, 0)
    gxl = np.linspace(-1.0, 1.0, W, dtype=f32)
    gyl = np.linspace(-1.0, 1.0, H, dtype=f32)
    basex = ((gxl[px_pos] + 1.0) * 0.5 * (W - 1)).astype(f32)
    basey = ((gyl[py_pos] + 1.0) * 0.5 * (H - 1)).astype(f32)
    lane = pos % 128
    tcol = pos // 128
    xgc2 = np.zeros((128, O * PT), f32)
    ygc2 = np.zeros((128, O * PT), f32)
    obase2 = np.zeros((128, O * PT), f32)
    for o in range(O):
        xgc2[lane, o * PT + tcol] = basex
        ygc2[lane, o * PT + tcol] = basey
        obase2[lane, o * PT + tcol] = o

    x32 = np.ascontiguousarray(
        np.asarray(inputs['query_map'], f32).reshape(B, DIM, HW)).astype(bf16)
    kv32 = np.ascontiguousarray(
        np.asarray(inputs['kv_map'], f32).reshape(B, DIM, HW)).astype(bf16)
    per_core = []
    for h in range(HEADS):
        wpack = np.zeros((DIM, O * 64), f32)
        for o in range(O):
            wpack[:, o * 64:o * 64 + 32] = Wk[h * HD:(h + 1) * HD, o * DIM:(o + 1) * DIM].T
            wpack[:, o * 64 + 32:o * 64 + 64] = Wv[h * HD:(h + 1) * HD, o * DIM:(o + 1) * DIM].T
        per_core.append({
            'x32': x32, 'kv32': kv32,
            'wfoldT': wfoldT, 'boff1c': boff1c,
            'woff2T': woff2T, 'boff2c': boff2c,
            'wqhT': np.ascontiguousarray(Wq[h * HD:(h + 1) * HD].T).astype(bf16),
            'wpack': wpack.astype(bf16),
            'wouthT': np.ascontiguousarray(Wout[:, h * HD:(h + 1) * HD].T).astype(bf16),
            'ebtab': np.ascontiguousarray(ebx[..., h]).reshape(1, W * 111 * W),
            'xgc2': xgc2, 'ygc2': ygc2, 'obase2': obase2,
        })
    return per_core


def _build_program():
    import os
    STAGE = int(os.environ.get('KSTAGE', '4'))
    SUB = int(os.environ.get('KSUB', '9'))
    import concourse.bass as bass
    import concourse.bacc as baccm
    import concourse.tile as tile
    from concourse import mybir
    from concourse.masks import make_identity
    import contextlib

    f32 = mybir.dt.float32
    b16d = mybir.dt.bfloat16
    i16 = mybir.dt.int16
    ALU = mybir.AluOpType
    ACTF = mybir.ActivationFunctionType

    nc = baccm.Bacc(trn_type="TRN2")

    x32 = nc.dram_tensor('x32', [B, DIM, HW], b16d, kind='ExternalInput')
    kv32 = nc.dram_tensor('kv32', [B, DIM, HW], b16d, kind='ExternalInput')
    wfoldT = nc.dram_tensor('wfoldT', [DIM, 64], b16d, kind='ExternalInput')
    boff1c = nc.dram_tensor('boff1c', [64, 1], f32, kind='ExternalInput')
    woff2T = nc.dram_tensor('woff2T', [64, 18], b16d, kind='ExternalInput')
    boff2c = nc.dram_tensor('boff2c', [18, 1], f32, kind='ExternalInput')
    wqhT = nc.dram_tensor('wqhT', [DIM, HD], b16d, kind='ExternalInput')
    wpack = nc.dram_tensor('wpack', [DIM, O * 64], b16d, kind='ExternalInput')
    wouthT = nc.dram_tensor('wouthT', [HD, DIM], b16d, kind='ExternalInput')
    ebtab = nc.dram_tensor('ebtab', [1, W * 111 * W], b16d, kind='ExternalInput')
    xgc2 = nc.dram_tensor('xgc2', [128, O * PT], f32, kind='ExternalInput')
    ygc2 = nc.dram_tensor('ygc2', [128, O * PT], f32, kind='ExternalInput')
    obase2 = nc.dram_tensor('obase2', [128, O * PT], f32, kind='ExternalInput')
    outp = nc.dram_tensor('outp', [B, DIM, HW], b16d, kind='ExternalOutput')

    offd = nc.dram_tensor('offd', [B, 18, HW], f32, kind='Internal')
    idxd = nc.dram_tensor('idxd', [B, O * NP], i16, kind='Internal')
    p4d = nc.dram_tensor('p4d', [B, NP * O * 256], b16d, kind='Internal')

    def dap(handle, off, dims):
        return bass.AP(tensor=handle, offset=off, ap=[list(d) for d in dims])

    with tile.TileContext(nc) as tc:
        ctx = contextlib.ExitStack()
        consts = ctx.enter_context(tc.tile_pool(name='consts', bufs=1))
        big = ctx.enter_context(tc.tile_pool(name='big', bufs=1))
        work = ctx.enter_context(tc.tile_pool(name='work', bufs=2))
        gpool = ctx.enter_context(tc.tile_pool(name='gpool', bufs=1))
        ntt = ctx.enter_context(tc.tile_pool(name='ntt', bufs=3))
        st_ps = ctx.enter_context(tc.tile_pool(name='st_ps', bufs=2, space='PSUM'))
        av_ps = ctx.enter_context(tc.tile_pool(name='av_ps', bufs=2, space='PSUM'))
        misc_ps = ctx.enter_context(tc.tile_pool(name='misc_ps', bufs=2, space='PSUM'))

        with ctx:
            wfoldT_s = consts.tile([128, 2, 64], b16d)
            nc.sync.dma_start(out=wfoldT_s, in_=wfoldT.rearrange('(c l) k -> l c k', c=2))
            boff1_s = consts.tile([64, 1], f32)
            nc.sync.dma_start(out=boff1_s, in_=boff1c[:, :])
            woff2T_s = consts.tile([64, 18], b16d)
            nc.sync.dma_start(out=woff2T_s, in_=woff2T[:, :])
            boff2_s = consts.tile([18, 1], f32)
            nc.sync.dma_start(out=boff2_s, in_=boff2c[:, :])
            wqhT_s = consts.tile([128, 2, HD], b16d)
            nc.sync.dma_start(out=wqhT_s, in_=wqhT.rearrange('(c l) k -> l c k', c=2))
            wpack_s = consts.tile([128, 2, O * 64], b16d)
            nc.sync.dma_start(out=wpack_s, in_=wpack.rearrange('(c l) k -> l c k', c=2))
            wouthT_s = consts.tile([HD, DIM], b16d)
            nc.sync.dma_start(out=wouthT_s, in_=wouthT[:, :])
            xgc2_s = consts.tile([128, O * PT], f32)
            nc.sync.dma_start(out=xgc2_s, in_=xgc2[:, :])
            ygc2_s = consts.tile([128, O * PT], f32)
            nc.sync.dma_start(out=ygc2_s, in_=ygc2[:, :])
            obase2_s = consts.tile([128, O * PT], f32)
            nc.sync.dma_start(out=obase2_s, in_=obase2[:, :])
            ident = consts.tile([128, 128], b16d)
            make_identity(nc, ident)
            ones32 = consts.tile([1, HD], f32)
            nc.vector.memset(ones32, 1.0)

            kvt_all = [big.tile([128, PT, 65], b16d, tag=f'kvt{b}', name=f'kvt{b}') for b in range(B)]
            k4_all = [big.tile([HD, PT * 128], b16d, tag=f'k4{b}', name=f'k4{b}') for b in range(B)]
            qs_all = [big.tile([HD, HW], b16d, tag=f'qs{b}', name=f'qs{b}') for b in range(B)]
            for b in range(B):
                nc.vector.memset(kvt_all[b][:, :, 64:65], 1.0)

            for b in range(B):
                x_bf = big.tile([128, 2, HW], b16d, tag='x_bf')
                kv_bf = big.tile([128, 2, HW], b16d, tag='kv_bf')
                for cc in range(2):
                    nc.sync.dma_start(out=x_bf[:, cc, :],
                                      in_=x32[b, cc * 128:(cc + 1) * 128, :])
                    nc.sync.dma_start(out=kv_bf[:, cc, :],
                                      in_=kv32[b, cc * 128:(cc + 1) * 128, :])

                # ---- offset MLP ----
                off_s = big.tile([18, HW], f32, tag='off')
                for ch in range(NCH):
                    sl = slice(ch * IC, (ch + 1) * IC)
                    hid_ps = misc_ps.tile([64, IC], f32, tag='mps')
                    for cc in range(2):
                        nc.tensor.matmul(hid_ps, wfoldT_s[:, cc, :], x_bf[:, cc, sl],
                                         start=(cc == 0), stop=(cc == 1))
                    hid_s = work.tile([64, IC], b16d, tag='hid')
                    nc.scalar.activation(hid_s, hid_ps, ACTF.Gelu_apprx_tanh, bias=boff1_s)
                    off_ps = misc_ps.tile([18, IC], f32, tag='mps')
                    nc.tensor.matmul(off_ps, woff2T_s, hid_s, start=True, stop=True)
                    nc.scalar.activation(off_s[:, sl], off_ps, ACTF.Identity, bias=boff2_s)
                nc.sync.dma_start(out=offd[b, :, :], in_=off_s)
                offx = work.tile([128, O * PT], f32, tag='offx')
                offy = work.tile([128, O * PT], f32, tag='offy')
                nc.vector.memset(offx, 0.0)
                nc.vector.memset(offy, 0.0)
                for xy, dst in ((0, offx), (1, offy)):
                    for o in range(O):
                        base = (b * 18 + xy * O + o) * HW
                        nc.sync.dma_start(
                            out=dst[:, o * PT:o * PT + 24],
                            in_=dap(offd, base, [[1, 128], [128, 24]]))
                        nc.sync.dma_start(
                            out=dst[:64, o * PT + 24:o * PT + 25],
                            in_=dap(offd, base + 24 * 128, [[1, 64], [128, 1]]))

                # ---- coords / weights / indices ----
                px = work.tile([128, O * PT], f32, tag='px')
                py = work.tile([128, O * PT], f32, tag='py')
                nc.vector.tensor_scalar(px, offx, float(OS * 0.5 * (W - 1)), None, ALU.mult)
                nc.vector.tensor_tensor(px, px, xgc2_s, ALU.add)
                nc.vector.tensor_scalar(px, px, 0.0, float(W - 1), ALU.max, ALU.min)
                nc.vector.tensor_scalar(py, offy, float(OS * 0.5 * (H - 1)), None, ALU.mult)
                nc.vector.tensor_tensor(py, py, ygc2_s, ALU.add)
                nc.vector.tensor_scalar(py, py, 0.0, float(H - 1), ALU.max, ALU.min)
                MAGIC = 12582912.0  # 1.5 * 2**23: float32 round-to-int trick
                x0f = work.tile([128, O * PT], f32, tag='x0f')
                y0f = work.tile([128, O * PT], f32, tag='y0f')
                nc.vector.tensor_scalar(x0f, px, -0.5, MAGIC, ALU.add, ALU.add)
                nc.vector.tensor_scalar(x0f, x0f, -MAGIC, None, ALU.add)
                nc.vector.tensor_scalar(y0f, py, -0.5, MAGIC, ALU.add, ALU.add)
                nc.vector.tensor_scalar(y0f, y0f, -MAGIC, None, ALU.add)
                wx = work.tile([128, O * PT], f32, tag='wx')
                wy = work.tile([128, O * PT], f32, tag='wy')
                nc.vector.tensor_tensor(wx, px, x0f, ALU.subtract)
                nc.vector.tensor_tensor(wy, py, y0f, ALU.subtract)
                r0 = work.tile([128, O * PT], f32, tag='r0')
                nc.vector.tensor_scalar(r0, y0f, float(W), None, ALU.mult)
                nc.vector.tensor_tensor(r0, r0, x0f, ALU.add)
                nc.vector.tensor_scalar(r0, r0, float(O), None, ALU.mult)
                nc.vector.tensor_tensor(r0, r0, obase2_s, ALU.add)
                nc.vector.tensor_scalar(r0, r0, 0.0, float(NP * O - 1), ALU.max, ALU.min)
                idx16 = work.tile([128, O * PT], i16, tag='idx')
                nc.vector.tensor_copy(idx16, r0)
                nc.sync.dma_start(
                    out=dap(idxd, b * O * NP, [[1, 128], [NP, O], [128, PT]]),
                    in_=idx16)
                idxw = big.tile([128, O * (NP // 16)], i16, tag='idxw')
                for rep in range(8):
                    nc.sync.dma_start(
                        out=idxw[rep * 16:(rep + 1) * 16, :],
                        in_=dap(idxd, b * O * NP,
                                [[1, 16], [NP, O], [16, NP // 16]]))
                w4 = big.tile([128, O * PT, 4], b16d, tag='w4')
                onemwx = work.tile([128, O * PT], f32, tag='o1x')
                onemwy = work.tile([128, O * PT], f32, tag='o1y')
                nc.vector.tensor_scalar(onemwx, wx, -1.0, 1.0, ALU.mult, ALU.add)
                nc.vector.tensor_scalar(onemwy, wy, -1.0, 1.0, ALU.mult, ALU.add)
                wtmp = work.tile([128, O * PT], f32, tag='wtmp')
                for n, (aa, cc2) in enumerate(((onemwx, onemwy), (wx, onemwy),
                                               (onemwx, wy), (wx, wy))):
                    nc.vector.tensor_tensor(wtmp, aa, cc2, ALU.mult)
                    nc.vector.tensor_copy(w4[:, :, n], wtmp)

                # ---- P projection ----
                if STAGE < 2:
                    continue
                p_sb = big.tile([128, 26, O * 64], b16d, tag='p_sb')
                nc.vector.memset(p_sb[:, 24:26, :], 0.0)
                for it in range(PT):
                    rows = 128 if it < 24 else HW - 24 * 128
                    for n2 in range(2):
                        pps = misc_ps.tile([128, 288], f32, tag='mps')
                        for cc in range(2):
                            nc.tensor.matmul(
                                pps[:rows],
                                kv_bf[:, cc, it * 128:it * 128 + rows],
                                wpack_s[:, cc, n2 * 288:(n2 + 1) * 288],
                                start=(cc == 0), stop=(cc == 1))
                        nc.scalar.activation(p_sb[:rows, it, n2 * 288:(n2 + 1) * 288],
                                             pps[:rows], ACTF.Copy)
                # ---- P4 build ----
                for n, dlt in enumerate((0, 1, 56, 57)):
                    lo = dlt
                    first = 128 - lo
                    for o in range(O):
                        base = b * NP * O * 256 + o * 256 + n * 64
                        nc.sync.dma_start(
                            out=dap(p4d, base,
                                    [[O * 256, first], [128 * O * 256, PT], [1, 64]]),
                            in_=p_sb[lo:128, 0:PT, o * 64:(o + 1) * 64])
                        if lo > 0:
                            nc.sync.dma_start(
                                out=dap(p4d, base + first * O * 256,
                                        [[O * 256, lo], [128 * O * 256, PT], [1, 64]]),
                                in_=p_sb[0:lo, 1:PT + 1, o * 64:(o + 1) * 64])

                # ---- gather + bilinear ----
                if STAGE < 3:
                    continue
                for o in range(O):
                    gt = gpool.tile([128, PT, 256], b16d, tag='gt', bufs=2)
                    nc.gpsimd.dma_gather(
                        out_ap=gt,
                        in_ap=dap(p4d, b * NP * O * 256, [[256, NP * O], [1, 256]]),
                        idxs_ap=idxw[:, o * (NP // 16):(o + 1) * (NP // 16)],
                        num_idxs=NP, num_idxs_reg=NP,
                        elem_size=256, elem_step=256, single_packet=False)
                    if SUB < 1:
                        continue
                    w4x = gpool.tile([128, PT, 4, 64], b16d, tag='w4x')
                    w4sl = w4[:, o * PT:(o + 1) * PT, :]
                    nc.gpsimd.tensor_copy(
                        w4x,
                        bass.AP(tensor=w4sl.tensor, offset=w4sl.offset,
                                ap=[*w4sl.ap, [0, 64]]))
                    if SUB < 2:
                        continue
                    nc.vector.tensor_tensor(
                        gt.rearrange('l t c -> l (t c)'),
                        gt.rearrange('l t c -> l (t c)'),
                        w4x.rearrange('l t n c -> l (t n c)'), ALU.mult)
                    gt4 = gt.rearrange('l t (n c) -> l t n c', c=64)
                    t12 = gpool.tile([128, PT, 2, 64], b16d, tag='t12')
                    nc.vector.tensor_tensor(t12, gt4[:, :, 0:2, :], gt4[:, :, 2:4, :],
                                            ALU.add)
                    if o == 0:
                        nc.vector.tensor_tensor(kvt_all[b][:, :, 0:64],
                                                t12[:, :, 0, :], t12[:, :, 1, :],
                                                ALU.add)
                    else:
                        nc.vector.tensor_tensor(kvt_all[b][:, :, 0:64],
                                                kvt_all[b][:, :, 0:64], t12[:, :, 0, :],
                                                ALU.add)
                        nc.vector.tensor_tensor(kvt_all[b][:, :, 0:64],
                                                kvt_all[b][:, :, 0:64], t12[:, :, 1, :],
                                                ALU.add)

                # ---- k transposes ----
                for t in (range(PT) if SUB >= 3 else []):
                    tp_ps = misc_ps.tile([HD, 128], b16d, tag='mps')
                    nc.tensor.transpose(tp_ps, kvt_all[b][:, t, 0:HD], ident)
                    nc.scalar.activation(k4_all[b][:, t * 128:(t + 1) * 128], tp_ps,
                                         ACTF.Copy)

                # ---- q_h scaled ----
                for ch in (range(NCH) if SUB >= 4 else []):
                    sl = slice(ch * IC, (ch + 1) * IC)
                    q_ps = misc_ps.tile([HD, IC], f32, tag='mps')
                    for cc in range(2):
                        nc.tensor.matmul(q_ps, wqhT_s[:, cc, :], x_bf[:, cc, sl],
                                         start=(cc == 0), stop=(cc == 1))
                    nc.scalar.activation(qs_all[b][:, sl], q_ps, ACTF.Copy,
                                         scale=float(SCALE))

            # ---------- attention ----------
            if STAGE < 4:
                for b in range(B):
                    dummy = work.tile([128, HW], b16d, tag='dmy')
                    nc.vector.memset(dummy, 0.25)
                    for cc in range(2):
                        nc.sync.dma_start(out=outp[b, cc * 128:(cc + 1) * 128, :],
                                          in_=dummy)
            jgroups = [(j, min(j + 2, PT)) for j in range(0, PT, 2)]
            for ic in (range(NCH) if STAGE >= 4 else []):
                yi0 = ic * 8
                avs = [av_ps.tile([33, IC], f32, tag=f'av{b}', bufs=1,
                                  name=f'av_{ic}_{b}') for b in range(B)]
                for (ja, jb) in jgroups:
                    jn = jb - ja
                    eb = ntt.tile([128, 2, IC], b16d, tag='eb')
                    for r in range(jn):
                        jt = ja + r
                        j0 = jt * 128
                        jend = min(j0 + 128, HW)
                        seg = j0
                        while seg < jend:
                            y_j = seg // W
                            seg_end = min((y_j + 1) * W, jend)
                            cnt = seg_end - seg
                            x_j0 = seg - y_j * W
                            basee = (x_j0 * 111 * W
                                     + (yi0 - y_j + (H - 1)) * W)
                            nc.sync.dma_start(
                                out=eb[seg - j0:seg - j0 + cnt, r, :]
                                    .rearrange('l (a c) -> l a c', a=8),
                                in_=dap(ebtab, basee,
                                        [[111 * W, cnt], [W, 8], [1, W]]))
                            seg = seg_end
                        if jend < j0 + 128:
                            nc.vector.memset(eb[jend - j0:128, r, :], 0.0)
                    for b in range(B):
                        stp = st_ps.tile([128, 2, 512], f32, tag='stp')
                        for r in range(jn):
                            jt = ja + r
                            nc.tensor.matmul(
                                stp[:, r, 0:IC],
                                k4_all[b][:, jt * 128:(jt + 1) * 128],
                                qs_all[b][:, ic * IC:(ic + 1) * IC],
                                start=True, stop=True)
                        nt = ntt.tile([128, 2, 512], b16d, tag='nt')
                        nc.scalar.activation(
                            nt[:, 0:jn, :].rearrange('l a c -> l (a c)'),
                            stp[:, 0:jn, :].rearrange('l a c -> l (a c)'),
                            ACTF.Exp)
                        nc.vector.tensor_tensor(
                            nt[:, 0:jn, 0:IC], nt[:, 0:jn, 0:IC],
                            eb[:, 0:jn, :], ALU.mult)
                        for r in range(jn):
                            jt = ja + r
                            nc.tensor.matmul(
                                avs[b], kvt_all[b][:, jt, 32:65], nt[:, r, 0:IC],
                                start=(jt == 0), stop=(jt == PT - 1),
                                skip_group_check=True)
                for b in range(B):
                    o1 = work.tile([33, IC], f32, tag='o1')
                    nc.scalar.activation(o1, avs[b], ACTF.Copy)
                    rc = work.tile([1, IC], f32, tag='rc')
                    nc.vector.reciprocal(rc, o1[32:33, :])
                    rb_ps = misc_ps.tile([HD, IC], f32, tag='mps')
                    nc.tensor.matmul(rb_ps, ones32, rc, start=True, stop=True)
                    att = work.tile([HD, IC], b16d, tag='att')
                    nc.vector.tensor_tensor(att, o1[0:HD, :], rb_ps, ALU.mult)
                    for mc in range(2):
                        wo_ps = misc_ps.tile([128, IC], f32, tag='mps')
                        nc.tensor.matmul(wo_ps, wouthT_s[:, mc * 128:(mc + 1) * 128],
                                         att, start=True, stop=True)
                        osb = work.tile([128, IC], b16d, tag='osb')
                        nc.scalar.activation(osb, wo_ps, ACTF.Copy)
                        nc.sync.dma_start(
                            out=outp[b, mc * 128:(mc + 1) * 128, ic * IC:(ic + 1) * IC],
                            in_=osb)
    nc.finalize()
    return nc


def _get_state():
    if 'nc' not in _STATE:
        _STATE['nc'] = _build_program()
    return _STATE['nc']


def kernel(query_map, kv_map, Wq, Wk, Wv, Woff1, boff1, Woff2, boff2,
           cpb_w1, cpb_b1, cpb_w2, cpb_b2, Wout, bout):
    from concourse.bass_utils import run_bass_kernel_spmd
    inputs = dict(query_map=query_map, kv_map=kv_map, Wq=Wq, Wk=Wk, Wv=Wv,
                  Woff1=Woff1, boff1=boff1, Woff2=Woff2, boff2=boff2,
                  cpb_w1=cpb_w1, cpb_b1=cpb_b1, cpb_w2=cpb_w2, cpb_b2=cpb_b2,
                  Wout=Wout, bout=bout)
    nc = _get_state()
    in_maps = _host_prep(inputs)
    res = run_bass_kernel_spmd(nc, in_maps, list(range(HEADS)))
    out = np.zeros((B, DIM, HW), np.float32)
    for c in range(HEADS):
        out += np.asarray(res.results[c]['outp']).astype(np.float32)
    out += np.asarray(bout, np.float32)[None, :, None]
    return out.reshape(B, DIM, H, W)


# revision 23
# speedup vs baseline: 1.6076x; 1.2023x over previous
"""Deformable cross-attention on 8 trn2 NeuronCores via Bass/Tile.

Sharding: core c owns head c (both batch elements); host sums the 8
per-head partials of the output projection.

Per core: offset MLP (Woff1@Wq folded on host), per-head per-offset
projected kv maps P via PE matmuls (bf16), one-descriptor-per-sample
bilinear gather from a quad-row P4 layout via dma_gather, flash-style
attention over key tiles with multiplicative exp(bias) windows, and the
per-head slice of the output projection.
"""

import numpy as np
import ml_dtypes

bf16 = ml_dtypes.bfloat16

B, DIM, H, W = 2, 256, 56, 56
HEADS, O, OS = 8, 9, 0.1
HD = DIM // HEADS          # 32
HW = H * W                 # 3136
SCALE = np.float32(HD ** -0.5)
PT = 25                    # position tiles of 128
NP = PT * 128              # 3200 padded positions
IC = 448                   # attention i-chunk (8 rows of y_i)
NCH = HW // IC             # 7
EBT = 111 * 111

_STATE: dict = {}


def _host_prep(inputs):
    f32 = np.float32
    Wq = np.asarray(inputs['Wq'], f32)
    Wk = np.asarray(inputs['Wk'], f32)
    Wv = np.asarray(inputs['Wv'], f32)
    Woff1 = np.asarray(inputs['Woff1'], f32)
    Woff2 = np.asarray(inputs['Woff2'], f32)
    Wout = np.asarray(inputs['Wout'], f32)
    wfoldT = np.ascontiguousarray((Woff1 @ Wq).T).astype(bf16)       # (256,64)
    woff2T = np.ascontiguousarray(Woff2.T).astype(bf16)              # (64,18)
    boff1c = np.asarray(inputs['boff1'], f32).reshape(64, 1)
    boff2c = np.asarray(inputs['boff2'], f32).reshape(18, 1)

    dyv = np.arange(-(H - 1), H, dtype=f32) / (H - 1)
    dxv = np.arange(-(W - 1), W, dtype=f32) / (W - 1)
    gy, gx = np.meshgrid(dyv, dxv, indexing='ij')
    coords = np.stack([gy, gx], -1).reshape(-1, 2).astype(f32)
    hdn = np.maximum(coords @ np.asarray(inputs['cpb_w1'], f32).T
                     + np.asarray(inputs['cpb_b1'], f32), 0.0)
    table = hdn @ np.asarray(inputs['cpb_w2'], f32).T + np.asarray(inputs['cpb_b2'], f32)
    ebt = np.exp(table.astype(f32))                                  # (12321, 8)
    # expand over x_j so EB window DMAs have all-positive steps:
    # ebx[h, x_j, a, x_i] = exp(T2)[a, x_i + 55 - x_j, h]
    eb2 = ebt.reshape(111, 111, HEADS)
    xj = np.arange(W)[:, None, None]
    aa = np.arange(111)[None, :, None]
    xi = np.arange(W)[None, None, :]
    ebx = eb2[aa, xi + (W - 1) - xj, :].astype(bf16)      # (56, 111, 56, 8)

    pos = np.arange(NP)
    py_pos = np.where(pos < HW, pos // W, 0)
    px_pos = np.where(pos < HW, pos % W, 0)
    gxl = np.linspace(-1.0, 1.0, W, dtype=f32)
    gyl = np.linspace(-1.0, 1.0, H, dtype=f32)
    basex = ((gxl[px_pos] + 1.0) * 0.5 * (W - 1)).astype(f32)
    basey = ((gyl[py_pos] + 1.0) * 0.5 * (H - 1)).astype(f32)
    lane = pos % 128
    tcol = pos // 128
    xgc2 = np.zeros((128, O * PT), f32)
    ygc2 = np.zeros((128, O * PT), f32)
    obase2 = np.zeros((128, O * PT), f32)
    for o in range(O):
        xgc2[lane, o * PT + tcol] = basex
        ygc2[lane, o * PT + tcol] = basey
        obase2[lane, o * PT + tcol] = o

    x32 = np.ascontiguousarray(
        np.asarray(inputs['query_map'], f32).reshape(B, DIM, HW)).astype(bf16)
    kv32 = np.ascontiguousarray(
        np.asarray(inputs['kv_map'], f32).reshape(B, DIM, HW)).astype(bf16)
    per_core = []
    for h in range(HEADS):
        wpack = np.zeros((DIM, O * 64), f32)
        for o in range(O):
            wpack[:, o * 64:o * 64 + 32] = Wk[h * HD:(h + 1) * HD, o * DIM:(o + 1) * DIM].T
            wpack[:, o * 64 + 32:o * 64 + 64] = Wv[h * HD:(h + 1) * HD, o * DIM:(o + 1) * DIM].T
        per_core.append({
            'x32': x32, 'kv32': kv32,
            'wfoldT': wfoldT, 'boff1c': boff1c,
            'woff2T': woff2T, 'boff2c': boff2c,
            'wqhT': np.ascontiguousarray(Wq[h * HD:(h + 1) * HD].T).astype(bf16),
            'wpack': wpack.astype(bf16),
            'wouthT': np.ascontiguousarray(Wout[:, h * HD:(h + 1) * HD].T).astype(bf16),
            'ebtab': np.ascontiguousarray(ebx[..., h]).reshape(1, W * 111 * W),
            'xgc2': xgc2, 'ygc2': ygc2, 'obase2': obase2,
        })
    return per_core


def _build_program():
    import os
    STAGE = int(os.environ.get('KSTAGE', '4'))
    SUB = int(os.environ.get('KSUB', '9'))
    import concourse.bass as bass
    import concourse.bacc as baccm
    import concourse.tile as tile
    from concourse import mybir
    from concourse.masks import make_identity
    import contextlib

    f32 = mybir.dt.float32
    b16d = mybir.dt.bfloat16
    i16 = mybir.dt.int16
    ALU = mybir.AluOpType
    ACTF = mybir.ActivationFunctionType

    nc = baccm.Bacc(trn_type="TRN2")

    x32 = nc.dram_tensor('x32', [B, DIM, HW], b16d, kind='ExternalInput')
    kv32 = nc.dram_tensor('kv32', [B, DIM, HW], b16d, kind='ExternalInput')
    wfoldT = nc.dram_tensor('wfoldT', [DIM, 64], b16d, kind='ExternalInput')
    boff1c = nc.dram_tensor('boff1c', [64, 1], f32, kind='ExternalInput')
    woff2T = nc.dram_tensor('woff2T', [64, 18], b16d, kind='ExternalInput')
    boff2c = nc.dram_tensor('boff2c', [18, 1], f32, kind='ExternalInput')
    wqhT = nc.dram_tensor('wqhT', [DIM, HD], b16d, kind='ExternalInput')
    wpack = nc.dram_tensor('wpack', [DIM, O * 64], b16d, kind='ExternalInput')
    wouthT = nc.dram_tensor('wouthT', [HD, DIM], b16d, kind='ExternalInput')
    ebtab = nc.dram_tensor('ebtab', [1, W * 111 * W], b16d, kind='ExternalInput')
    xgc2 = nc.dram_tensor('xgc2', [128, O * PT], f32, kind='ExternalInput')
    ygc2 = nc.dram_tensor('ygc2', [128, O * PT], f32, kind='ExternalInput')
    obase2 = nc.dram_tensor('obase2', [128, O * PT], f32, kind='ExternalInput')
    outp = nc.dram_tensor('outp', [B, DIM, HW], b16d, kind='ExternalOutput')

    offd = nc.dram_tensor('offd', [B, 18, HW], f32, kind='Internal')
    idxd = nc.dram_tensor('idxd', [B, O * NP], i16, kind='Internal')
    p4d = nc.dram_tensor('p4d', [B, NP * O * 256], b16d, kind='Internal')

    def dap(handle, off, dims):
        return bass.AP(tensor=handle, offset=off, ap=[list(d) for d in dims])

    with tile.TileContext(nc) as tc:
        ctx = contextlib.ExitStack()
        consts = ctx.enter_context(tc.tile_pool(name='consts', bufs=1))
        big = ctx.enter_context(tc.tile_pool(name='big', bufs=1))
        work = ctx.enter_context(tc.tile_pool(name='work', bufs=2))
        gpool = ctx.enter_context(tc.tile_pool(name='gpool', bufs=1))
        ntt = ctx.enter_context(tc.tile_pool(name='ntt', bufs=3))
        st_ps = ctx.enter_context(tc.tile_pool(name='st_ps', bufs=2, space='PSUM'))
        av_ps = ctx.enter_context(tc.tile_pool(name='av_ps', bufs=2, space='PSUM'))
        misc_ps = ctx.enter_context(tc.tile_pool(name='misc_ps', bufs=2, space='PSUM'))

        with ctx:
            wfoldT_s = consts.tile([128, 2, 64], b16d)
            nc.sync.dma_start(out=wfoldT_s, in_=wfoldT.rearrange('(c l) k -> l c k', c=2))
            boff1_s = consts.tile([64, 1], f32)
            nc.sync.dma_start(out=boff1_s, in_=boff1c[:, :])
            woff2T_s = consts.tile([64, 18], b16d)
            nc.sync.dma_start(out=woff2T_s, in_=woff2T[:, :])
            boff2_s = consts.tile([18, 1], f32)
            nc.sync.dma_start(out=boff2_s, in_=boff2c[:, :])
            wqhT_s = consts.tile([128, 2, HD], b16d)
            nc.sync.dma_start(out=wqhT_s, in_=wqhT.rearrange('(c l) k -> l c k', c=2))
            wpack_s = consts.tile([128, 2, O * 64], b16d)
            nc.sync.dma_start(out=wpack_s, in_=wpack.rearrange('(c l) k -> l c k', c=2))
            wouthT_s = consts.tile([HD, DIM], b16d)
            nc.sync.dma_start(out=wouthT_s, in_=wouthT[:, :])
            xgc2_s = consts.tile([128, O * PT], f32)
            nc.sync.dma_start(out=xgc2_s, in_=xgc2[:, :])
            ygc2_s = consts.tile([128, O * PT], f32)
            nc.sync.dma_start(out=ygc2_s, in_=ygc2[:, :])
            obase2_s = consts.tile([128, O * PT], f32)
            nc.sync.dma_start(out=obase2_s, in_=obase2[:, :])
            ident = consts.tile([128, 128], b16d)
            make_identity(nc, ident)
            ones32 = consts.tile([1, HD], f32)
            nc.vector.memset(ones32, 1.0)

            kvt_all = [big.tile([128, PT, 65], b16d, tag=f'kvt{b}', name=f'kvt{b}') for b in range(B)]
            k4_all = [big.tile([HD, PT * 128], b16d, tag=f'k4{b}', name=f'k4{b}') for b in range(B)]
            qs_all = [big.tile([HD, HW], b16d, tag=f'qs{b}', name=f'qs{b}') for b in range(B)]
            for b in range(B):
                nc.vector.memset(kvt_all[b][:, :, 64:65], 1.0)

            for b in range(B):
                x_bf = big.tile([128, 2, HW], b16d, tag='x_bf')
                kv_bf = big.tile([128, 2, HW], b16d, tag='kv_bf')
                for cc in range(2):
                    nc.sync.dma_start(out=x_bf[:, cc, :],
                                      in_=x32[b, cc * 128:(cc + 1) * 128, :])
                    nc.sync.dma_start(out=kv_bf[:, cc, :],
                                      in_=kv32[b, cc * 128:(cc + 1) * 128, :])

                # ---- offset MLP ----
                off_s = big.tile([18, HW], f32, tag='off')
                for ch in range(NCH):
                    sl = slice(ch * IC, (ch + 1) * IC)
                    hid_ps = misc_ps.tile([64, IC], f32, tag='mps')
                    for cc in range(2):
                        nc.tensor.matmul(hid_ps, wfoldT_s[:, cc, :], x_bf[:, cc, sl],
                                         start=(cc == 0), stop=(cc == 1))
                    hid_s = work.tile([64, IC], b16d, tag='hid')
                    nc.scalar.activation(hid_s, hid_ps, ACTF.Gelu_apprx_tanh, bias=boff1_s)
                    off_ps = misc_ps.tile([18, IC], f32, tag='mps')
                    nc.tensor.matmul(off_ps, woff2T_s, hid_s, start=True, stop=True)
                    nc.scalar.activation(off_s[:, sl], off_ps, ACTF.Identity, bias=boff2_s)
                nc.sync.dma_start(out=offd[b, :, :], in_=off_s)
                offx = work.tile([128, O * PT], f32, tag='offx')
                offy = work.tile([128, O * PT], f32, tag='offy')
                nc.vector.memset(offx, 0.0)
                nc.vector.memset(offy, 0.0)
                for xy, dst in ((0, offx), (1, offy)):
                    for o in range(O):
                        base = (b * 18 + xy * O + o) * HW
                        nc.sync.dma_start(
                            out=dst[:, o * PT:o * PT + 24],
                            in_=dap(offd, base, [[1, 128], [128, 24]]))
                        nc.sync.dma_start(
                            out=dst[:64, o * PT + 24:o * PT + 25],
                            in_=dap(offd, base + 24 * 128, [[1, 64], [128, 1]]))

                # ---- coords / weights / indices ----
                px = work.tile([128, O * PT], f32, tag='px')
                py = work.tile([128, O * PT], f32, tag='py')
                nc.vector.tensor_scalar(px, offx, float(OS * 0.5 * (W - 1)), None, ALU.mult)
                nc.vector.tensor_tensor(px, px, xgc2_s, ALU.add)
                nc.vector.tensor_scalar(px, px, 0.0, float(W - 1), ALU.max, ALU.min)
                nc.vector.tensor_scalar(py, offy, float(OS * 0.5 * (H - 1)), None, ALU.mult)
                nc.vector.tensor_tensor(py, py, ygc2_s, ALU.add)
                nc.vector.tensor_scalar(py, py, 0.0, float(H - 1), ALU.max, ALU.min)
                MAGIC = 12582912.0  # 1.5 * 2**23: float32 round-to-int trick
                x0f = work.tile([128, O * PT], f32, tag='x0f')
                y0f = work.tile([128, O * PT], f32, tag='y0f')
                nc.vector.tensor_scalar(x0f, px, -0.5, MAGIC, ALU.add, ALU.add)
                nc.vector.tensor_scalar(x0f, x0f, -MAGIC, None, ALU.add)
                nc.vector.tensor_scalar(y0f, py, -0.5, MAGIC, ALU.add, ALU.add)
                nc.vector.tensor_scalar(y0f, y0f, -MAGIC, None, ALU.add)
                wx = work.tile([128, O * PT], f32, tag='wx')
                wy = work.tile([128, O * PT], f32, tag='wy')
                nc.vector.tensor_tensor(wx, px, x0f, ALU.subtract)
                nc.vector.tensor_tensor(wy, py, y0f, ALU.subtract)
                r0 = work.tile([128, O * PT], f32, tag='r0')
                nc.vector.tensor_scalar(r0, y0f, float(W), None, ALU.mult)
                nc.vector.tensor_tensor(r0, r0, x0f, ALU.add)
                nc.vector.tensor_scalar(r0, r0, float(O), None, ALU.mult)
                nc.vector.tensor_tensor(r0, r0, obase2_s, ALU.add)
                nc.vector.tensor_scalar(r0, r0, 0.0, float(NP * O - 1), ALU.max, ALU.min)
                idx16 = work.tile([128, O * PT], i16, tag='idx')
                nc.vector.tensor_copy(idx16, r0)
                nc.sync.dma_start(
                    out=dap(idxd, b * O * NP, [[1, 128], [NP, O], [128, PT]]),
                    in_=idx16)
                idxw = big.tile([128, O * (NP // 16)], i16, tag='idxw')
                for rep in range(8):
                    nc.sync.dma_start(
                        out=idxw[rep * 16:(rep + 1) * 16, :],
                        in_=dap(idxd, b * O * NP,
                                [[1, 16], [NP, O], [16, NP // 16]]))
                w4 = big.tile([128, O * PT, 4], b16d, tag='w4')
                onemwx = work.tile([128, O * PT], f32, tag='o1x')
                onemwy = work.tile([128, O * PT], f32, tag='o1y')
                nc.vector.tensor_scalar(onemwx, wx, -1.0, 1.0, ALU.mult, ALU.add)
                nc.vector.tensor_scalar(onemwy, wy, -1.0, 1.0, ALU.mult, ALU.add)
                wtmp = work.tile([128, O * PT], f32, tag='wtmp')
                for n, (aa, cc2) in enumerate(((onemwx, onemwy), (wx, onemwy),
                                               (onemwx, wy), (wx, wy))):
                    nc.vector.tensor_tensor(wtmp, aa, cc2, ALU.mult)
                    nc.vector.tensor_copy(w4[:, :, n], wtmp)

                # ---- P projection ----
                if STAGE < 2:
                    continue
                p_sb = big.tile([128, 26, O * 64], b16d, tag='p_sb')
                nc.vector.memset(p_sb[:, 24:26, :], 0.0)
                for it in range(PT):
                    rows = 128 if it < 24 else HW - 24 * 128
                    for n2 in range(2):
                        pps = misc_ps.tile([128, 288], f32, tag='mps')
                        for cc in range(2):
                            nc.tensor.matmul(
                                pps[:rows],
                                kv_bf[:, cc, it * 128:it * 128 + rows],
                                wpack_s[:, cc, n2 * 288:(n2 + 1) * 288],
                                start=(cc == 0), stop=(cc == 1))
                        nc.scalar.activation(p_sb[:rows, it, n2 * 288:(n2 + 1) * 288],
                                             pps[:rows], ACTF.Copy)
                # ---- P4 build ----
                for n, dlt in enumerate((0, 1, 56, 57)):
                    lo = dlt
                    first = 128 - lo
                    for o in range(O):
                        base = b * NP * O * 256 + o * 256 + n * 64
                        nc.sync.dma_start(
                            out=dap(p4d, base,
                                    [[O * 256, first], [128 * O * 256, PT], [1, 64]]),
                            in_=p_sb[lo:128, 0:PT, o * 64:(o + 1) * 64])
                        if lo > 0:
                            nc.sync.dma_start(
                                out=dap(p4d, base + first * O * 256,
                                        [[O * 256, lo], [128 * O * 256, PT], [1, 64]]),
                                in_=p_sb[0:lo, 1:PT + 1, o * 64:(o + 1) * 64])

                # ---- gather + bilinear ----
                if STAGE < 3:
                    continue
                for o in range(O):
                    gt = gpool.tile([128, PT, 256], b16d, tag='gt', bufs=2)
                    nc.gpsimd.dma_gather(
                        out_ap=gt,
                        in_ap=dap(p4d, b * NP * O * 256, [[256, NP * O], [1, 256]]),
                        idxs_ap=idxw[:, o * (NP // 16):(o + 1) * (NP // 16)],
                        num_idxs=NP, num_idxs_reg=NP,
                        elem_size=256, elem_step=256, single_packet=False)
                    if SUB < 1:
                        continue
                    w4x = gpool.tile([128, PT, 4, 64], b16d, tag='w4x')
                    w4sl = w4[:, o * PT:(o + 1) * PT, :]
                    nc.gpsimd.tensor_copy(
                        w4x,
                        bass.AP(tensor=w4sl.tensor, offset=w4sl.offset,
                                ap=[*w4sl.ap, [0, 64]]))
                    if SUB < 2:
                        continue
                    nc.vector.tensor_tensor(
                        gt.rearrange('l t c -> l (t c)'),
                        gt.rearrange('l t c -> l (t c)'),
                        w4x.rearrange('l t n c -> l (t n c)'), ALU.mult)
                    gt4 = gt.rearrange('l t (n c) -> l t n c', c=64)
                    t12 = gpool.tile([128, PT, 2, 64], b16d, tag='t12')
                    nc.vector.tensor_tensor(t12, gt4[:, :, 0:2, :], gt4[:, :, 2:4, :],
                                            ALU.add)
                    if o == 0:
                        nc.vector.tensor_tensor(kvt_all[b][:, :, 0:64],
                                                t12[:, :, 0, :], t12[:, :, 1, :],
                                                ALU.add)
                    else:
                        nc.vector.tensor_tensor(kvt_all[b][:, :, 0:64],
                                                kvt_all[b][:, :, 0:64], t12[:, :, 0, :],
                                                ALU.add)
                        nc.vector.tensor_tensor(kvt_all[b][:, :, 0:64],
                                                kvt_all[b][:, :, 0:64], t12[:, :, 1, :],
                                                ALU.add)

                # ---- k transposes ----
                for t in (range(PT) if SUB >= 3 else []):
                    tp_ps = misc_ps.tile([HD, 128], b16d, tag='mps')
                    nc.tensor.transpose(tp_ps, kvt_all[b][:, t, 0:HD], ident)
                    nc.scalar.activation(k4_all[b][:, t * 128:(t + 1) * 128], tp_ps,
                                         ACTF.Copy)

                # ---- q_h scaled ----
                for ch in (range(NCH) if SUB >= 4 else []):
                    sl = slice(ch * IC, (ch + 1) * IC)
                    q_ps = misc_ps.tile([HD, IC], f32, tag='mps')
                    for cc in range(2):
                        nc.tensor.matmul(q_ps, wqhT_s[:, cc, :], x_bf[:, cc, sl],
                                         start=(cc == 0), stop=(cc == 1))
                    nc.scalar.activation(qs_all[b][:, sl], q_ps, ACTF.Copy,
                                         scale=float(SCALE))

            # ---------- attention ----------
            if STAGE < 4:
                for b in range(B):
                    dummy = work.tile([128, HW], b16d, tag='dmy')
                    nc.vector.memset(dummy, 0.25)
                    for cc in range(2):
                        nc.sync.dma_start(out=outp[b, cc * 128:(cc + 1) * 128, :],
                                          in_=dummy)
            jgroups = [(j, min(j + 2, PT)) for j in range(0, PT, 2)]
            for ic in (range(NCH) if STAGE >= 4 else []):
                yi0 = ic * 8
                avs = [av_ps.tile([33, IC], f32, tag=f'av{b}', bufs=1,
                                  name=f'av_{ic}_{b}') for b in range(B)]
                for (ja, jb) in jgroups:
                    jn = jb - ja
                    eb = ntt.tile([128, 2, IC], b16d, tag='eb')
                    for r in range(jn):
                        jt = ja + r
                        j0 = jt * 128
                        jend = min(j0 + 128, HW)
                        seg = j0
                        while seg < jend:
                            y_j = seg // W
                            seg_end = min((y_j + 1) * W, jend)
                            cnt = seg_end - seg
                            x_j0 = seg - y_j * W
                            basee = (x_j0 * 111 * W
                                     + (yi0 - y_j + (H - 1)) * W)
                            nc.sync.dma_start(
                                out=eb[seg - j0:seg - j0 + cnt, r, :]
                                    .rearrange('l (a c) -> l a c', a=8),
                                in_=dap(ebtab, basee,
                                        [[111 * W, cnt], [W, 8], [1, W]]))
                            seg = seg_end
                        if jend < j0 + 128:
                            nc.vector.memset(eb[jend - j0:128, r, :], 0.0)
                    for b in range(B):
                        stp = st_ps.tile([128, 2, 512], f32, tag='stp')
                        for r in range(jn):
                            jt = ja + r
                            nc.tensor.matmul(
                                stp[:, r, 0:IC],
                                k4_all[b][:, jt * 128:(jt + 1) * 128],
                                qs_all[b][:, ic * IC:(ic + 1) * IC],
                                start=True, stop=True)
                        nt = ntt.tile([128, 2, 512], b16d, tag='nt')
                        nc.scalar.activation(
                            nt[:, 0:jn, :].rearrange('l a c -> l (a c)'),
                            stp[:, 0:jn, :].rearrange('l a c -> l (a c)'),
                            ACTF.Exp)
                        nc.vector.tensor_tensor(
                            nt[:, 0:jn, 0:IC], nt[:, 0:jn, 0:IC],
                            eb[:, 0:jn, :], ALU.mult)
                        for r in range(jn):
                            jt = ja + r
                            nc.tensor.matmul(
                                avs[b], kvt_all[b][:, jt, 32:65], nt[:, r, 0:IC],
                                start=(jt == 0), stop=(jt == PT - 1),
                                skip_group_check=True)
                for b in range(B):
                    o1 = work.tile([33, IC], f32, tag='o1')
                    nc.scalar.activation(o1, avs[b], ACTF.Copy)
                    rc = work.tile([1, IC], f32, tag='rc')
                    nc.vector.reciprocal(rc, o1[32:33, :])
                    rb_ps = misc_ps.tile([HD, IC], f32, tag='mps')
                    nc.tensor.matmul(rb_ps, ones32, rc, start=True, stop=True)
                    att = work.tile([HD, IC], b16d, tag='att')
                    nc.vector.tensor_tensor(att, o1[0:HD, :], rb_ps, ALU.mult)
                    for mc in range(2):
                        wo_ps = misc_ps.tile([128, IC], f32, tag='mps')
                        nc.tensor.matmul(wo_ps, wouthT_s[:, mc * 128:(mc + 1) * 128],
                                         att, start=True, stop=True)
                        osb = work.tile([128, IC], b16d, tag='osb')
                        nc.scalar.activation(osb, wo_ps, ACTF.Copy)
                        nc.sync.dma_start(
                            out=outp[b, mc * 128:(mc + 1) * 128, ic * IC:(ic + 1) * IC],
                            in_=osb)
    nc.finalize()
    return nc


def _get_state():
    if 'nc' not in _STATE:
        _STATE['nc'] = _build_program()
    return _STATE['nc']


def _make_runner(nc):
    """Build (once) a cached jitted shard_map executable for the bass module.

    Mirrors concourse.bass2jax.run_bass_via_pjrt's multi-core path, but
    keeps the jitted callable across kernel() calls so only data transfer
    and execution happen per call.
    """
    import jax
    import numpy as _np
    from jax.sharding import Mesh, PartitionSpec
    from jax.experimental.shard_map import shard_map
    import concourse.mybir as mybir
    from concourse.bass2jax import (_bass_exec_p, install_neuronx_cc_hook,
                                    partition_id_tensor)

    install_neuronx_cc_hook()
    partition_name = nc.partition_id_tensor.name if nc.partition_id_tensor else None
    in_names, out_names, out_avals, zero_shapes = [], [], [], []
    for alloc in nc.m.functions[0].allocations:
        if not isinstance(alloc, mybir.MemoryLocationSet):
            continue
        name = alloc.memorylocations[0].name
        if alloc.kind == 'ExternalInput':
            if name != partition_name:
                in_names.append(name)
        elif alloc.kind == 'ExternalOutput':
            out_names.append(name)
            shape = tuple(alloc.tensor_shape)
            dtype = mybir.dt.np(alloc.dtype)
            out_avals.append(jax.core.ShapedArray(shape, dtype))
            zero_shapes.append((shape, dtype))
    n_params = len(in_names)
    n_outs = len(out_avals)
    all_in_names = list(in_names) + list(out_names)
    if partition_name is not None:
        all_in_names.append(partition_name)
    donate = tuple(range(n_params, n_params + n_outs))

    def _body(*args):
        operands = list(args)
        if partition_name is not None:
            operands.append(partition_id_tensor())
        return tuple(_bass_exec_p.bind(
            *operands, out_avals=tuple(out_avals), in_names=tuple(all_in_names),
            out_names=tuple(out_names), lowering_input_output_aliases=(),
            sim_require_finite=True, sim_require_nnan=True, nc=nc))

    devices = jax.devices()[:HEADS]
    mesh = Mesh(_np.asarray(devices), ('core',))
    in_specs = (PartitionSpec('core'),) * (n_params + n_outs)
    out_specs = (PartitionSpec('core'),) * n_outs
    sharded = jax.jit(
        shard_map(_body, mesh=mesh, in_specs=in_specs, out_specs=out_specs,
                  check_rep=False),
        donate_argnums=donate, keep_unused=True)

    def run(in_maps):
        concat_in = [
            _np.concatenate([_np.asarray(in_maps[c][nm]) for c in range(HEADS)], axis=0)
            for nm in in_names]
        concat_zeros = [_np.zeros((HEADS * sh[0], *sh[1:]), dt)
                        for sh, dt in zero_shapes]
        out_arrs = sharded(*concat_in, *concat_zeros)
        return {nm: _np.asarray(out_arrs[i]) for i, nm in enumerate(out_names)}

    return run


def kernel(query_map, kv_map, Wq, Wk, Wv, Woff1, boff1, Woff2, boff2,
           cpb_w1, cpb_b1, cpb_w2, cpb_b2, Wout, bout):
    inputs = dict(query_map=query_map, kv_map=kv_map, Wq=Wq, Wk=Wk, Wv=Wv,
                  Woff1=Woff1, boff1=boff1, Woff2=Woff2, boff2=boff2,
                  cpb_w1=cpb_w1, cpb_b1=cpb_b1, cpb_w2=cpb_w2, cpb_b2=cpb_b2,
                  Wout=Wout, bout=bout)
    nc = _get_state()
    if 'runner' not in _STATE:
        _STATE['runner'] = _make_runner(nc)
    in_maps = _host_prep(inputs)
    outs = _STATE['runner'](in_maps)
    part = outs['outp'].reshape(HEADS, B, DIM, HW).astype(np.float32)
    out = part.sum(axis=0)
    out += np.asarray(bout, np.float32)[None, :, None]
    return out.reshape(B, DIM, H, W)


# revision 24
# speedup vs baseline: 1.7088x; 1.0629x over previous
"""Deformable cross-attention on 8 trn2 NeuronCores via Bass/Tile.

Sharding: core c owns head c (both batch elements); host sums the 8
per-head partials of the output projection.

Per core: offset MLP (Woff1@Wq folded on host), per-head per-offset
projected kv maps P via PE matmuls (bf16), one-descriptor-per-sample
bilinear gather from a quad-row P4 layout via dma_gather, flash-style
attention over key tiles with multiplicative exp(bias) windows, and the
per-head slice of the output projection.
"""

import numpy as np
import ml_dtypes

bf16 = ml_dtypes.bfloat16

B, DIM, H, W = 2, 256, 56, 56
HEADS, O, OS = 8, 9, 0.1
HD = DIM // HEADS          # 32
HW = H * W                 # 3136
SCALE = np.float32(HD ** -0.5)
PT = 25                    # position tiles of 128
NP = PT * 128              # 3200 padded positions
IC = 448                   # attention i-chunk (8 rows of y_i)
NCH = HW // IC             # 7
EBT = 111 * 111

_STATE: dict = {}


def _host_prep(inputs):
    f32 = np.float32
    Wq = np.asarray(inputs['Wq'], f32)
    Wk = np.asarray(inputs['Wk'], f32)
    Wv = np.asarray(inputs['Wv'], f32)
    Woff1 = np.asarray(inputs['Woff1'], f32)
    Woff2 = np.asarray(inputs['Woff2'], f32)
    Wout = np.asarray(inputs['Wout'], f32)
    wfoldT = np.ascontiguousarray((Woff1 @ Wq).T).astype(bf16)       # (256,64)
    woff2T = np.ascontiguousarray(Woff2.T).astype(bf16)              # (64,18)
    boff1c = np.asarray(inputs['boff1'], f32).reshape(64, 1)
    boff2c = np.asarray(inputs['boff2'], f32).reshape(18, 1)

    dyv = np.arange(-(H - 1), H, dtype=f32) / (H - 1)
    dxv = np.arange(-(W - 1), W, dtype=f32) / (W - 1)
    gy, gx = np.meshgrid(dyv, dxv, indexing='ij')
    coords = np.stack([gy, gx], -1).reshape(-1, 2).astype(f32)
    hdn = np.maximum(coords @ np.asarray(inputs['cpb_w1'], f32).T
                     + np.asarray(inputs['cpb_b1'], f32), 0.0)
    table = hdn @ np.asarray(inputs['cpb_w2'], f32).T + np.asarray(inputs['cpb_b2'], f32)
    ebt = np.exp(table.astype(f32))                                  # (12321, 8)
    # expand over x_j so EB window DMAs have all-positive steps:
    # ebx[h, x_j, a, x_i] = exp(T2)[a, x_i + 55 - x_j, h]
    eb2 = ebt.reshape(111, 111, HEADS)
    xj = np.arange(W)[:, None, None]
    aa = np.arange(111)[None, :, None]
    xi = np.arange(W)[None, None, :]
    ebx = eb2[aa, xi + (W - 1) - xj, :].astype(bf16)      # (56, 111, 56, 8)

    pos = np.arange(NP)
    py_pos = np.where(pos < HW, pos // W, 0)
    px_pos = np.where(pos < HW, pos % W, 0)
    gxl = np.linspace(-1.0, 1.0, W, dtype=f32)
    gyl = np.linspace(-1.0, 1.0, H, dtype=f32)
    basex = ((gxl[px_pos] + 1.0) * 0.5 * (W - 1)).astype(f32)
    basey = ((gyl[py_pos] + 1.0) * 0.5 * (H - 1)).astype(f32)
    lane = pos % 128
    tcol = pos // 128
    xgc2 = np.zeros((128, O * PT), f32)
    ygc2 = np.zeros((128, O * PT), f32)
    obase2 = np.zeros((128, O * PT), f32)
    for o in range(O):
        xgc2[lane, o * PT + tcol] = basex
        ygc2[lane, o * PT + tcol] = basey
        obase2[lane, o * PT + tcol] = o

    x32 = np.ascontiguousarray(
        np.asarray(inputs['query_map'], f32).reshape(B, DIM, HW)).astype(bf16)
    kv32 = np.ascontiguousarray(
        np.asarray(inputs['kv_map'], f32).reshape(B, DIM, HW)).astype(bf16)
    per_core = []
    for h in range(HEADS):
        wpack = np.zeros((DIM, O * 64), f32)
        for o in range(O):
            wpack[:, o * 64:o * 64 + 32] = Wk[h * HD:(h + 1) * HD, o * DIM:(o + 1) * DIM].T
            wpack[:, o * 64 + 32:o * 64 + 64] = Wv[h * HD:(h + 1) * HD, o * DIM:(o + 1) * DIM].T
        per_core.append({
            'x32': x32, 'kv32': kv32,
            'wfoldT': wfoldT, 'boff1c': boff1c,
            'woff2T': woff2T, 'boff2c': boff2c,
            'wqhT': np.ascontiguousarray(Wq[h * HD:(h + 1) * HD].T).astype(bf16),
            'wpack': wpack.astype(bf16),
            'wouthT': np.ascontiguousarray(Wout[:, h * HD:(h + 1) * HD].T).astype(bf16),
            'ebtab': np.ascontiguousarray(ebx[..., h]).reshape(1, W * 111 * W),
            'xgc2': xgc2, 'ygc2': ygc2, 'obase2': obase2,
        })
    return per_core


def _build_program():
    import os
    STAGE = int(os.environ.get('KSTAGE', '4'))
    SUB = int(os.environ.get('KSUB', '9'))
    import concourse.bass as bass
    import concourse.bacc as baccm
    import concourse.tile as tile
    from concourse import mybir
    from concourse.masks import make_identity
    import contextlib

    f32 = mybir.dt.float32
    b16d = mybir.dt.bfloat16
    i16 = mybir.dt.int16
    ALU = mybir.AluOpType
    ACTF = mybir.ActivationFunctionType

    nc = baccm.Bacc(trn_type="TRN2")

    x32 = nc.dram_tensor('x32', [B, DIM, HW], b16d, kind='ExternalInput')
    kv32 = nc.dram_tensor('kv32', [B, DIM, HW], b16d, kind='ExternalInput')
    wfoldT = nc.dram_tensor('wfoldT', [DIM, 64], b16d, kind='ExternalInput')
    boff1c = nc.dram_tensor('boff1c', [64, 1], f32, kind='ExternalInput')
    woff2T = nc.dram_tensor('woff2T', [64, 18], b16d, kind='ExternalInput')
    boff2c = nc.dram_tensor('boff2c', [18, 1], f32, kind='ExternalInput')
    wqhT = nc.dram_tensor('wqhT', [DIM, HD], b16d, kind='ExternalInput')
    wpack = nc.dram_tensor('wpack', [DIM, O * 64], b16d, kind='ExternalInput')
    wouthT = nc.dram_tensor('wouthT', [HD, DIM], b16d, kind='ExternalInput')
    ebtab = nc.dram_tensor('ebtab', [1, W * 111 * W], b16d, kind='ExternalInput')
    xgc2 = nc.dram_tensor('xgc2', [128, O * PT], f32, kind='ExternalInput')
    ygc2 = nc.dram_tensor('ygc2', [128, O * PT], f32, kind='ExternalInput')
    obase2 = nc.dram_tensor('obase2', [128, O * PT], f32, kind='ExternalInput')
    outp = nc.dram_tensor('outp', [B, DIM, HW], b16d, kind='ExternalOutput')

    offd = nc.dram_tensor('offd', [B, 18, HW], f32, kind='Internal')
    idxd = nc.dram_tensor('idxd', [B, O * NP], i16, kind='Internal')
    p4d = nc.dram_tensor('p4d', [B, NP * O * 256], b16d, kind='Internal')

    def dap(handle, off, dims):
        return bass.AP(tensor=handle, offset=off, ap=[list(d) for d in dims])

    with tile.TileContext(nc) as tc:
        ctx = contextlib.ExitStack()
        consts = ctx.enter_context(tc.tile_pool(name='consts', bufs=1))
        big = ctx.enter_context(tc.tile_pool(name='big', bufs=1))
        work = ctx.enter_context(tc.tile_pool(name='work', bufs=2))
        gpool = ctx.enter_context(tc.tile_pool(name='gpool', bufs=1))
        ntt = ctx.enter_context(tc.tile_pool(name='ntt', bufs=3))
        st_ps = ctx.enter_context(tc.tile_pool(name='st_ps', bufs=2, space='PSUM'))
        av_ps = ctx.enter_context(tc.tile_pool(name='av_ps', bufs=2, space='PSUM'))
        misc_ps = ctx.enter_context(tc.tile_pool(name='misc_ps', bufs=2, space='PSUM'))

        with ctx:
            wfoldT_s = consts.tile([128, 2, 64], b16d)
            nc.sync.dma_start(out=wfoldT_s, in_=wfoldT.rearrange('(c l) k -> l c k', c=2))
            boff1_s = consts.tile([64, 1], f32)
            nc.sync.dma_start(out=boff1_s, in_=boff1c[:, :])
            woff2T_s = consts.tile([64, 18], b16d)
            nc.sync.dma_start(out=woff2T_s, in_=woff2T[:, :])
            boff2_s = consts.tile([18, 1], f32)
            nc.sync.dma_start(out=boff2_s, in_=boff2c[:, :])
            wqhT_s = consts.tile([128, 2, HD], b16d)
            nc.sync.dma_start(out=wqhT_s, in_=wqhT.rearrange('(c l) k -> l c k', c=2))
            wpack_s = consts.tile([128, 2, O * 64], b16d)
            nc.sync.dma_start(out=wpack_s, in_=wpack.rearrange('(c l) k -> l c k', c=2))
            wouthT_s = consts.tile([HD, DIM], b16d)
            nc.sync.dma_start(out=wouthT_s, in_=wouthT[:, :])
            xgc2_s = consts.tile([128, O * PT], f32)
            nc.sync.dma_start(out=xgc2_s, in_=xgc2[:, :])
            ygc2_s = consts.tile([128, O * PT], f32)
            nc.sync.dma_start(out=ygc2_s, in_=ygc2[:, :])
            obase2_s = consts.tile([128, O * PT], f32)
            nc.sync.dma_start(out=obase2_s, in_=obase2[:, :])
            ident = consts.tile([128, 128], b16d)
            make_identity(nc, ident)
            ones32 = consts.tile([1, HD], f32)
            nc.vector.memset(ones32, 1.0)

            kvt_all = [big.tile([128, PT, 65], b16d, tag=f'kvt{b}', name=f'kvt{b}') for b in range(B)]
            k4_all = [big.tile([HD, PT * 128], b16d, tag=f'k4{b}', name=f'k4{b}') for b in range(B)]
            qs_all = [big.tile([HD, HW], b16d, tag=f'qs{b}', name=f'qs{b}') for b in range(B)]
            for b in range(B):
                nc.vector.memset(kvt_all[b][:, :, 64:65], 1.0)

            for b in range(B):
                x_bf = big.tile([128, 2, HW], b16d, tag='x_bf')
                kv_bf = big.tile([128, 2, HW], b16d, tag='kv_bf')
                for cc in range(2):
                    nc.sync.dma_start(out=x_bf[:, cc, :],
                                      in_=x32[b, cc * 128:(cc + 1) * 128, :])
                    nc.sync.dma_start(out=kv_bf[:, cc, :],
                                      in_=kv32[b, cc * 128:(cc + 1) * 128, :])

                # ---- offset MLP ----
                off_s = big.tile([18, HW], f32, tag='off')
                for ch in range(NCH):
                    sl = slice(ch * IC, (ch + 1) * IC)
                    hid_ps = misc_ps.tile([64, IC], f32, tag='mps')
                    for cc in range(2):
                        nc.tensor.matmul(hid_ps, wfoldT_s[:, cc, :], x_bf[:, cc, sl],
                                         start=(cc == 0), stop=(cc == 1))
                    hid_s = work.tile([64, IC], b16d, tag='hid')
                    nc.scalar.activation(hid_s, hid_ps, ACTF.Gelu_apprx_tanh, bias=boff1_s)
                    off_ps = misc_ps.tile([18, IC], f32, tag='mps')
                    nc.tensor.matmul(off_ps, woff2T_s, hid_s, start=True, stop=True)
                    nc.scalar.activation(off_s[:, sl], off_ps, ACTF.Identity, bias=boff2_s)
                nc.sync.dma_start(out=offd[b, :, :], in_=off_s)
                offx = work.tile([128, O * PT], f32, tag='offx')
                offy = work.tile([128, O * PT], f32, tag='offy')
                nc.vector.memset(offx, 0.0)
                nc.vector.memset(offy, 0.0)
                for xy, dst in ((0, offx), (1, offy)):
                    for o in range(O):
                        base = (b * 18 + xy * O + o) * HW
                        nc.sync.dma_start(
                            out=dst[:, o * PT:o * PT + 24],
                            in_=dap(offd, base, [[1, 128], [128, 24]]))
                        nc.sync.dma_start(
                            out=dst[:64, o * PT + 24:o * PT + 25],
                            in_=dap(offd, base + 24 * 128, [[1, 64], [128, 1]]))

                # ---- coords / weights / indices ----
                px = work.tile([128, O * PT], f32, tag='px')
                py = work.tile([128, O * PT], f32, tag='py')
                nc.vector.tensor_scalar(px, offx, float(OS * 0.5 * (W - 1)), None, ALU.mult)
                nc.vector.tensor_tensor(px, px, xgc2_s, ALU.add)
                nc.vector.tensor_scalar(px, px, 0.0, float(W - 1), ALU.max, ALU.min)
                nc.vector.tensor_scalar(py, offy, float(OS * 0.5 * (H - 1)), None, ALU.mult)
                nc.vector.tensor_tensor(py, py, ygc2_s, ALU.add)
                nc.vector.tensor_scalar(py, py, 0.0, float(H - 1), ALU.max, ALU.min)
                MAGIC = 12582912.0  # 1.5 * 2**23: float32 round-to-int trick
                x0f = work.tile([128, O * PT], f32, tag='x0f')
                y0f = work.tile([128, O * PT], f32, tag='y0f')
                nc.vector.tensor_scalar(x0f, px, -0.5, MAGIC, ALU.add, ALU.add)
                nc.vector.tensor_scalar(x0f, x0f, -MAGIC, None, ALU.add)
                nc.vector.tensor_scalar(y0f, py, -0.5, MAGIC, ALU.add, ALU.add)
                nc.vector.tensor_scalar(y0f, y0f, -MAGIC, None, ALU.add)
                wx = work.tile([128, O * PT], f32, tag='wx')
                wy = work.tile([128, O * PT], f32, tag='wy')
                nc.vector.tensor_tensor(wx, px, x0f, ALU.subtract)
                nc.vector.tensor_tensor(wy, py, y0f, ALU.subtract)
                r0 = work.tile([128, O * PT], f32, tag='r0')
                nc.vector.tensor_scalar(r0, y0f, float(W), None, ALU.mult)
                nc.vector.tensor_tensor(r0, r0, x0f, ALU.add)
                nc.vector.tensor_scalar(r0, r0, float(O), None, ALU.mult)
                nc.vector.tensor_tensor(r0, r0, obase2_s, ALU.add)
                nc.vector.tensor_scalar(r0, r0, 0.0, float(NP * O - 1), ALU.max, ALU.min)
                idx16 = work.tile([128, O * PT], i16, tag='idx')
                nc.vector.tensor_copy(idx16, r0)
                nc.sync.dma_start(
                    out=dap(idxd, b * O * NP, [[1, 128], [NP, O], [128, PT]]),
                    in_=idx16)
                idxw = big.tile([128, O * (NP // 16)], i16, tag='idxw')
                for rep in range(8):
                    nc.sync.dma_start(
                        out=idxw[rep * 16:(rep + 1) * 16, :],
                        in_=dap(idxd, b * O * NP,
                                [[1, 16], [NP, O], [16, NP // 16]]))
                w4 = big.tile([128, O * PT, 4], b16d, tag='w4')
                onemwx = work.tile([128, O * PT], f32, tag='o1x')
                onemwy = work.tile([128, O * PT], f32, tag='o1y')
                nc.vector.tensor_scalar(onemwx, wx, -1.0, 1.0, ALU.mult, ALU.add)
                nc.vector.tensor_scalar(onemwy, wy, -1.0, 1.0, ALU.mult, ALU.add)
                wtmp = work.tile([128, O * PT], f32, tag='wtmp')
                for n, (aa, cc2) in enumerate(((onemwx, onemwy), (wx, onemwy),
                                               (onemwx, wy), (wx, wy))):
                    nc.vector.tensor_tensor(wtmp, aa, cc2, ALU.mult)
                    nc.vector.tensor_copy(w4[:, :, n], wtmp)

                # ---- P projection ----
                if STAGE < 2:
                    continue
                p_sb = big.tile([128, 26, O * 64], b16d, tag='p_sb')
                nc.vector.memset(p_sb[:, 24:26, :], 0.0)
                for it in range(PT):
                    rows = 128 if it < 24 else HW - 24 * 128
                    for n2 in range(2):
                        pps = misc_ps.tile([128, 288], f32, tag='mps')
                        for cc in range(2):
                            nc.tensor.matmul(
                                pps[:rows],
                                kv_bf[:, cc, it * 128:it * 128 + rows],
                                wpack_s[:, cc, n2 * 288:(n2 + 1) * 288],
                                start=(cc == 0), stop=(cc == 1))
                        nc.scalar.activation(p_sb[:rows, it, n2 * 288:(n2 + 1) * 288],
                                             pps[:rows], ACTF.Copy)
                # ---- P4 build ----
                for n, dlt in enumerate((0, 1, 56, 57)):
                    lo = dlt
                    first = 128 - lo
                    for o in range(O):
                        base = b * NP * O * 256 + o * 256 + n * 64
                        nc.sync.dma_start(
                            out=dap(p4d, base,
                                    [[O * 256, first], [128 * O * 256, PT], [1, 64]]),
                            in_=p_sb[lo:128, 0:PT, o * 64:(o + 1) * 64])
                        if lo > 0:
                            nc.sync.dma_start(
                                out=dap(p4d, base + first * O * 256,
                                        [[O * 256, lo], [128 * O * 256, PT], [1, 64]]),
                                in_=p_sb[0:lo, 1:PT + 1, o * 64:(o + 1) * 64])

                # ---- gather + bilinear ----
                if STAGE < 3:
                    continue
                for o in range(O):
                    gt = gpool.tile([128, PT, 256], b16d, tag='gt', bufs=2)
                    nc.gpsimd.dma_gather(
                        out_ap=gt,
                        in_ap=dap(p4d, b * NP * O * 256, [[256, NP * O], [1, 256]]),
                        idxs_ap=idxw[:, o * (NP // 16):(o + 1) * (NP // 16)],
                        num_idxs=NP, num_idxs_reg=NP,
                        elem_size=256, elem_step=256, single_packet=False)
                    if SUB < 1:
                        continue
                    w4x = gpool.tile([128, PT, 4, 64], b16d, tag='w4x')
                    w4sl = w4[:, o * PT:(o + 1) * PT, :]
                    nc.gpsimd.tensor_copy(
                        w4x,
                        bass.AP(tensor=w4sl.tensor, offset=w4sl.offset,
                                ap=[*w4sl.ap, [0, 64]]))
                    if SUB < 2:
                        continue
                    nc.vector.tensor_tensor(
                        gt.rearrange('l t c -> l (t c)'),
                        gt.rearrange('l t c -> l (t c)'),
                        w4x.rearrange('l t n c -> l (t n c)'), ALU.mult)
                    gt4 = gt.rearrange('l t (n c) -> l t n c', c=64)
                    t12 = gpool.tile([128, PT, 2, 64], b16d, tag='t12')
                    nc.vector.tensor_tensor(t12, gt4[:, :, 0:2, :], gt4[:, :, 2:4, :],
                                            ALU.add)
                    if o == 0:
                        nc.vector.tensor_tensor(kvt_all[b][:, :, 0:64],
                                                t12[:, :, 0, :], t12[:, :, 1, :],
                                                ALU.add)
                    else:
                        nc.vector.tensor_tensor(kvt_all[b][:, :, 0:64],
                                                kvt_all[b][:, :, 0:64], t12[:, :, 0, :],
                                                ALU.add)
                        nc.vector.tensor_tensor(kvt_all[b][:, :, 0:64],
                                                kvt_all[b][:, :, 0:64], t12[:, :, 1, :],
                                                ALU.add)

                # ---- k transposes ----
                for t in (range(PT) if SUB >= 3 else []):
                    tp_ps = misc_ps.tile([HD, 128], b16d, tag='mps')
                    nc.tensor.transpose(tp_ps, kvt_all[b][:, t, 0:HD], ident)
                    nc.scalar.activation(k4_all[b][:, t * 128:(t + 1) * 128], tp_ps,
                                         ACTF.Copy)

                # ---- q_h scaled ----
                for ch in (range(NCH) if SUB >= 4 else []):
                    sl = slice(ch * IC, (ch + 1) * IC)
                    q_ps = misc_ps.tile([HD, IC], f32, tag='mps')
                    for cc in range(2):
                        nc.tensor.matmul(q_ps, wqhT_s[:, cc, :], x_bf[:, cc, sl],
                                         start=(cc == 0), stop=(cc == 1))
                    nc.scalar.activation(qs_all[b][:, sl], q_ps, ACTF.Copy,
                                         scale=float(SCALE))

            # ---------- attention ----------
            if STAGE < 4:
                for b in range(B):
                    dummy = work.tile([128, HW], b16d, tag='dmy')
                    nc.vector.memset(dummy, 0.25)
                    for cc in range(2):
                        nc.sync.dma_start(out=outp[b, cc * 128:(cc + 1) * 128, :],
                                          in_=dummy)
            jgroups = [(j, min(j + 2, PT)) for j in range(0, PT, 2)]
            for ic in (range(NCH) if STAGE >= 4 else []):
                yi0 = ic * 8
                avs = [av_ps.tile([33, IC], f32, tag=f'av{b}', bufs=1,
                                  name=f'av_{ic}_{b}') for b in range(B)]
                for (ja, jb) in jgroups:
                    jn = jb - ja
                    eb = ntt.tile([128, 2, IC], b16d, tag='eb')
                    for r in range(jn):
                        jt = ja + r
                        j0 = jt * 128
                        jend = min(j0 + 128, HW)
                        seg = j0
                        while seg < jend:
                            y_j = seg // W
                            seg_end = min((y_j + 1) * W, jend)
                            cnt = seg_end - seg
                            x_j0 = seg - y_j * W
                            basee = (x_j0 * 111 * W
                                     + (yi0 - y_j + (H - 1)) * W)
                            nc.sync.dma_start(
                                out=eb[seg - j0:seg - j0 + cnt, r, :]
                                    .rearrange('l (a c) -> l a c', a=8),
                                in_=dap(ebtab, basee,
                                        [[111 * W, cnt], [W, 8], [1, W]]))
                            seg = seg_end
                        if jend < j0 + 128:
                            nc.vector.memset(eb[jend - j0:128, r, :], 0.0)
                    for b in range(B):
                        stp = st_ps.tile([128, 2, 512], f32, tag='stp')
                        for r in range(jn):
                            jt = ja + r
                            nc.tensor.matmul(
                                stp[:, r, 0:IC],
                                k4_all[b][:, jt * 128:(jt + 1) * 128],
                                qs_all[b][:, ic * IC:(ic + 1) * IC],
                                start=True, stop=True)
                        nt = ntt.tile([128, 2, 512], b16d, tag='nt')
                        nc.scalar.activation(
                            nt[:, 0:jn, :].rearrange('l a c -> l (a c)'),
                            stp[:, 0:jn, :].rearrange('l a c -> l (a c)'),
                            ACTF.Exp)
                        nc.vector.tensor_tensor(
                            nt[:, 0:jn, 0:IC], nt[:, 0:jn, 0:IC],
                            eb[:, 0:jn, :], ALU.mult)
                        for r in range(jn):
                            jt = ja + r
                            nc.tensor.matmul(
                                avs[b], kvt_all[b][:, jt, 32:65], nt[:, r, 0:IC],
                                start=(jt == 0), stop=(jt == PT - 1),
                                skip_group_check=True)
                for b in range(B):
                    o1 = work.tile([33, IC], f32, tag='o1')
                    nc.scalar.activation(o1, avs[b], ACTF.Copy)
                    rc = work.tile([1, IC], f32, tag='rc')
                    nc.vector.reciprocal(rc, o1[32:33, :])
                    rb_ps = misc_ps.tile([HD, IC], f32, tag='mps')
                    nc.tensor.matmul(rb_ps, ones32, rc, start=True, stop=True)
                    att = work.tile([HD, IC], b16d, tag='att')
                    nc.vector.tensor_tensor(att, o1[0:HD, :], rb_ps, ALU.mult)
                    for mc in range(2):
                        wo_ps = misc_ps.tile([128, IC], f32, tag='mps')
                        nc.tensor.matmul(wo_ps, wouthT_s[:, mc * 128:(mc + 1) * 128],
                                         att, start=True, stop=True)
                        osb = work.tile([128, IC], b16d, tag='osb')
                        nc.scalar.activation(osb, wo_ps, ACTF.Copy)
                        nc.sync.dma_start(
                            out=outp[b, mc * 128:(mc + 1) * 128, ic * IC:(ic + 1) * IC],
                            in_=osb)
    nc.finalize()
    return nc


def _get_state():
    if 'nc' not in _STATE:
        _STATE['nc'] = _build_program()
    return _STATE['nc']


def _make_runner(nc):
    """Build (once) a cached jitted shard_map executable for the bass module.

    Mirrors concourse.bass2jax.run_bass_via_pjrt's multi-core path, but
    keeps the jitted callable across kernel() calls so only data transfer
    and execution happen per call.
    """
    import jax
    import numpy as _np
    from jax.sharding import Mesh, PartitionSpec
    from jax.experimental.shard_map import shard_map
    import concourse.mybir as mybir
    from concourse.bass2jax import (_bass_exec_p, install_neuronx_cc_hook,
                                    partition_id_tensor)

    install_neuronx_cc_hook()
    partition_name = nc.partition_id_tensor.name if nc.partition_id_tensor else None
    in_names, out_names, out_avals, zero_shapes = [], [], [], []
    for alloc in nc.m.functions[0].allocations:
        if not isinstance(alloc, mybir.MemoryLocationSet):
            continue
        name = alloc.memorylocations[0].name
        if alloc.kind == 'ExternalInput':
            if name != partition_name:
                in_names.append(name)
        elif alloc.kind == 'ExternalOutput':
            out_names.append(name)
            shape = tuple(alloc.tensor_shape)
            dtype = mybir.dt.np(alloc.dtype)
            out_avals.append(jax.core.ShapedArray(shape, dtype))
            zero_shapes.append((shape, dtype))
    n_params = len(in_names)
    n_outs = len(out_avals)
    all_in_names = list(in_names) + list(out_names)
    if partition_name is not None:
        all_in_names.append(partition_name)
    donate = tuple(range(n_params, n_params + n_outs))

    def _body(*args):
        operands = list(args)
        if partition_name is not None:
            operands.append(partition_id_tensor())
        return tuple(_bass_exec_p.bind(
            *operands, out_avals=tuple(out_avals), in_names=tuple(all_in_names),
            out_names=tuple(out_names), lowering_input_output_aliases=(),
            sim_require_finite=True, sim_require_nnan=True, nc=nc))

    devices = jax.devices()[:HEADS]
    mesh = Mesh(_np.asarray(devices), ('core',))
    shared = {'x32', 'kv32', 'wfoldT', 'boff1c', 'woff2T', 'boff2c',
              'xgc2', 'ygc2', 'obase2'}
    in_specs = tuple(PartitionSpec() if nm in shared else PartitionSpec('core')
                     for nm in in_names) + (PartitionSpec('core'),) * n_outs
    out_specs = (PartitionSpec('core'),) * n_outs
    sharded = jax.jit(
        shard_map(_body, mesh=mesh, in_specs=in_specs, out_specs=out_specs,
                  check_rep=False),
        donate_argnums=donate, keep_unused=True)

    def run(in_maps):
        concat_in = [
            _np.asarray(in_maps[0][nm]) if nm in shared else
            _np.concatenate([_np.asarray(in_maps[c][nm]) for c in range(HEADS)], axis=0)
            for nm in in_names]
        concat_zeros = [_np.zeros((HEADS * sh[0], *sh[1:]), dt)
                        for sh, dt in zero_shapes]
        out_arrs = sharded(*concat_in, *concat_zeros)
        return {nm: _np.asarray(out_arrs[i]) for i, nm in enumerate(out_names)}

    return run


def kernel(query_map, kv_map, Wq, Wk, Wv, Woff1, boff1, Woff2, boff2,
           cpb_w1, cpb_b1, cpb_w2, cpb_b2, Wout, bout):
    inputs = dict(query_map=query_map, kv_map=kv_map, Wq=Wq, Wk=Wk, Wv=Wv,
                  Woff1=Woff1, boff1=boff1, Woff2=Woff2, boff2=boff2,
                  cpb_w1=cpb_w1, cpb_b1=cpb_b1, cpb_w2=cpb_w2, cpb_b2=cpb_b2,
                  Wout=Wout, bout=bout)
    nc = _get_state()
    if 'runner' not in _STATE:
        _STATE['runner'] = _make_runner(nc)
    in_maps = _host_prep(inputs)
    outs = _STATE['runner'](in_maps)
    part = outs['outp'].reshape(HEADS, B, DIM, HW).astype(np.float32)
    out = part.sum(axis=0)
    out += np.asarray(bout, np.float32)[None, :, None]
    return out.reshape(B, DIM, H, W)


# revision 26
# speedup vs baseline: 1.8660x; 1.0920x over previous
"""Deformable cross-attention on 8 trn2 NeuronCores via Bass/Tile.

Sharding: core c owns head c (both batch elements); host sums the 8
per-head partials of the output projection.

Per core: offset MLP (Woff1@Wq folded on host), per-head per-offset
projected kv maps P via PE matmuls (bf16), one-descriptor-per-sample
bilinear gather from a quad-row P4 layout via dma_gather, flash-style
attention over key tiles with multiplicative exp(bias) windows, and the
per-head slice of the output projection.
"""

import numpy as np
import ml_dtypes

bf16 = ml_dtypes.bfloat16

B, DIM, H, W = 2, 256, 56, 56
HEADS, O, OS = 8, 9, 0.1
HD = DIM // HEADS          # 32
HW = H * W                 # 3136
SCALE = np.float32(HD ** -0.5)
PT = 25                    # position tiles of 128
NP = PT * 128              # 3200 padded positions
IC = 448                   # attention i-chunk (8 rows of y_i)
NCH = HW // IC             # 7
EBT = 111 * 111

_STATE: dict = {}


def _host_prep(inputs):
    f32 = np.float32
    Wq = np.asarray(inputs['Wq'], f32)
    Wk = np.asarray(inputs['Wk'], f32)
    Wv = np.asarray(inputs['Wv'], f32)
    Woff1 = np.asarray(inputs['Woff1'], f32)
    Woff2 = np.asarray(inputs['Woff2'], f32)
    Wout = np.asarray(inputs['Wout'], f32)
    wfoldT = np.ascontiguousarray((Woff1 @ Wq).T).astype(bf16)       # (256,64)
    woff2T = np.ascontiguousarray(Woff2.T).astype(bf16)              # (64,18)
    boff1c = np.asarray(inputs['boff1'], f32).reshape(64, 1)
    boff2c = np.asarray(inputs['boff2'], f32).reshape(18, 1)

    dyv = np.arange(-(H - 1), H, dtype=f32) / (H - 1)
    dxv = np.arange(-(W - 1), W, dtype=f32) / (W - 1)
    gy, gx = np.meshgrid(dyv, dxv, indexing='ij')
    coords = np.stack([gy, gx], -1).reshape(-1, 2).astype(f32)
    hdn = np.maximum(coords @ np.asarray(inputs['cpb_w1'], f32).T
                     + np.asarray(inputs['cpb_b1'], f32), 0.0)
    table = hdn @ np.asarray(inputs['cpb_w2'], f32).T + np.asarray(inputs['cpb_b2'], f32)
    ebt = np.exp(table.astype(f32))                                  # (12321, 8)
    # expand over x_j so EB window DMAs have all-positive steps:
    # ebx[h, x_j, a, x_i] = exp(T2)[a, x_i + 55 - x_j, h]
    eb2 = ebt.reshape(111, 111, HEADS)
    xj = np.arange(W)[:, None, None]
    aa = np.arange(111)[None, :, None]
    xi = np.arange(W)[None, None, :]
    ebx = eb2[aa, xi + (W - 1) - xj, :].astype(bf16)      # (56, 111, 56, 8)

    pos = np.arange(NP)
    py_pos = np.where(pos < HW, pos // W, 0)
    px_pos = np.where(pos < HW, pos % W, 0)
    gxl = np.linspace(-1.0, 1.0, W, dtype=f32)
    gyl = np.linspace(-1.0, 1.0, H, dtype=f32)
    basex = ((gxl[px_pos] + 1.0) * 0.5 * (W - 1)).astype(f32)
    basey = ((gyl[py_pos] + 1.0) * 0.5 * (H - 1)).astype(f32)
    lane = pos % 128
    tcol = pos // 128
    xgc2 = np.zeros((128, O * PT), f32)
    ygc2 = np.zeros((128, O * PT), f32)
    obase2 = np.zeros((128, O * PT), f32)
    for o in range(O):
        xgc2[lane, o * PT + tcol] = basex
        ygc2[lane, o * PT + tcol] = basey
        obase2[lane, o * PT + tcol] = o

    x32 = np.ascontiguousarray(
        np.asarray(inputs['query_map'], f32).reshape(B, DIM, HW)).astype(bf16)
    kv32 = np.ascontiguousarray(
        np.asarray(inputs['kv_map'], f32).reshape(B, DIM, HW)).astype(bf16)
    per_core = []
    for h in range(HEADS):
        wpack = np.zeros((DIM, O * 64), f32)
        for o in range(O):
            wpack[:, o * 64:o * 64 + 32] = Wk[h * HD:(h + 1) * HD, o * DIM:(o + 1) * DIM].T
            wpack[:, o * 64 + 32:o * 64 + 64] = Wv[h * HD:(h + 1) * HD, o * DIM:(o + 1) * DIM].T
        per_core.append({
            'x32': x32, 'kv32': kv32,
            'wfoldT': wfoldT, 'boff1c': boff1c,
            'woff2T': woff2T, 'boff2c': boff2c,
            'wqhT': np.ascontiguousarray(Wq[h * HD:(h + 1) * HD].T).astype(bf16),
            'wpack': wpack.astype(bf16),
            'wouthT': np.ascontiguousarray(Wout[:, h * HD:(h + 1) * HD].T).astype(bf16),
            'ebtab': np.ascontiguousarray(ebx[..., h]).reshape(1, W * 111 * W),
            'xgc2': xgc2, 'ygc2': ygc2, 'obase2': obase2,
        })
    return per_core


def _build_program():
    import os
    STAGE = int(os.environ.get('KSTAGE', '4'))
    SUB = int(os.environ.get('KSUB', '9'))
    import concourse.bass as bass
    import concourse.bacc as baccm
    import concourse.tile as tile
    from concourse import mybir
    from concourse.masks import make_identity
    import contextlib

    f32 = mybir.dt.float32
    b16d = mybir.dt.bfloat16
    i16 = mybir.dt.int16
    ALU = mybir.AluOpType
    ACTF = mybir.ActivationFunctionType

    nc = baccm.Bacc(trn_type="TRN2")

    x32 = nc.dram_tensor('x32', [B, DIM, HW], b16d, kind='ExternalInput')
    kv32 = nc.dram_tensor('kv32', [B, DIM, HW], b16d, kind='ExternalInput')
    wfoldT = nc.dram_tensor('wfoldT', [DIM, 64], b16d, kind='ExternalInput')
    boff1c = nc.dram_tensor('boff1c', [64, 1], f32, kind='ExternalInput')
    woff2T = nc.dram_tensor('woff2T', [64, 18], b16d, kind='ExternalInput')
    boff2c = nc.dram_tensor('boff2c', [18, 1], f32, kind='ExternalInput')
    wqhT = nc.dram_tensor('wqhT', [DIM, HD], b16d, kind='ExternalInput')
    wpack = nc.dram_tensor('wpack', [DIM, O * 64], b16d, kind='ExternalInput')
    wouthT = nc.dram_tensor('wouthT', [HD, DIM], b16d, kind='ExternalInput')
    ebtab = nc.dram_tensor('ebtab', [1, W * 111 * W], b16d, kind='ExternalInput')
    xgc2 = nc.dram_tensor('xgc2', [128, O * PT], f32, kind='ExternalInput')
    ygc2 = nc.dram_tensor('ygc2', [128, O * PT], f32, kind='ExternalInput')
    obase2 = nc.dram_tensor('obase2', [128, O * PT], f32, kind='ExternalInput')
    outp = nc.dram_tensor('outp', [B, DIM, HW], b16d, kind='ExternalOutput')

    offd = nc.dram_tensor('offd', [B, 18, HW], f32, kind='Internal')
    idxd = nc.dram_tensor('idxd', [B, O * NP], i16, kind='Internal')
    p4d = nc.dram_tensor('p4d', [B, NP * O * 256], b16d, kind='Internal')

    def dap(handle, off, dims):
        return bass.AP(tensor=handle, offset=off, ap=[list(d) for d in dims])

    with tile.TileContext(nc) as tc:
        ctx = contextlib.ExitStack()
        consts = ctx.enter_context(tc.tile_pool(name='consts', bufs=1))
        big = ctx.enter_context(tc.tile_pool(name='big', bufs=1))
        work = ctx.enter_context(tc.tile_pool(name='work', bufs=2))
        gpool = ctx.enter_context(tc.tile_pool(name='gpool', bufs=1))
        ntt = ctx.enter_context(tc.tile_pool(name='ntt', bufs=3))
        st_ps = ctx.enter_context(tc.tile_pool(name='st_ps', bufs=2, space='PSUM'))
        av_ps = ctx.enter_context(tc.tile_pool(name='av_ps', bufs=2, space='PSUM'))
        misc_ps = ctx.enter_context(tc.tile_pool(name='misc_ps', bufs=2, space='PSUM'))

        with ctx:
            wfoldT_s = consts.tile([128, 2, 64], b16d)
            nc.sync.dma_start(out=wfoldT_s, in_=wfoldT.rearrange('(c l) k -> l c k', c=2))
            boff1_s = consts.tile([64, 1], f32)
            nc.sync.dma_start(out=boff1_s, in_=boff1c[:, :])
            woff2T_s = consts.tile([64, 18], b16d)
            nc.sync.dma_start(out=woff2T_s, in_=woff2T[:, :])
            boff2_s = consts.tile([18, 1], f32)
            nc.sync.dma_start(out=boff2_s, in_=boff2c[:, :])
            wqhT_s = consts.tile([128, 2, HD], b16d)
            nc.sync.dma_start(out=wqhT_s, in_=wqhT.rearrange('(c l) k -> l c k', c=2))
            wpack_s = consts.tile([128, 2, O * 64], b16d)
            nc.sync.dma_start(out=wpack_s, in_=wpack.rearrange('(c l) k -> l c k', c=2))
            wouthT_s = consts.tile([HD, DIM], b16d)
            nc.sync.dma_start(out=wouthT_s, in_=wouthT[:, :])
            xgc2_s = consts.tile([128, O * PT], f32)
            nc.sync.dma_start(out=xgc2_s, in_=xgc2[:, :])
            ygc2_s = consts.tile([128, O * PT], f32)
            nc.sync.dma_start(out=ygc2_s, in_=ygc2[:, :])
            obase2_s = consts.tile([128, O * PT], f32)
            nc.sync.dma_start(out=obase2_s, in_=obase2[:, :])
            ident = consts.tile([128, 128], b16d)
            make_identity(nc, ident)
            ones32 = consts.tile([1, HD], f32)
            nc.vector.memset(ones32, 1.0)

            kvt_all = [big.tile([128, PT, 65], b16d, tag=f'kvt{b}', name=f'kvt{b}') for b in range(B)]
            k4_all = [big.tile([HD, PT * 128], b16d, tag=f'k4{b}', name=f'k4{b}') for b in range(B)]
            qs_all = [big.tile([HD, HW], b16d, tag=f'qs{b}', name=f'qs{b}') for b in range(B)]
            for b in range(B):
                nc.vector.memset(kvt_all[b][:, :, 64:65], 1.0)

            for b in range(B):
                x_bf = big.tile([128, 2, HW], b16d, tag='x_bf')
                kv_bf = big.tile([128, 2, HW], b16d, tag='kv_bf')
                for cc in range(2):
                    nc.sync.dma_start(out=x_bf[:, cc, :],
                                      in_=x32[b, cc * 128:(cc + 1) * 128, :])
                    nc.sync.dma_start(out=kv_bf[:, cc, :],
                                      in_=kv32[b, cc * 128:(cc + 1) * 128, :])

                # ---- offset MLP ----
                off_s = big.tile([18, HW], f32, tag='off')
                for ch in range(NCH):
                    sl = slice(ch * IC, (ch + 1) * IC)
                    hid_ps = misc_ps.tile([64, IC], f32, tag='mps')
                    for cc in range(2):
                        nc.tensor.matmul(hid_ps, wfoldT_s[:, cc, :], x_bf[:, cc, sl],
                                         start=(cc == 0), stop=(cc == 1))
                    hid_s = work.tile([64, IC], b16d, tag='hid')
                    nc.scalar.activation(hid_s, hid_ps, ACTF.Gelu_apprx_tanh, bias=boff1_s)
                    off_ps = misc_ps.tile([18, IC], f32, tag='mps')
                    nc.tensor.matmul(off_ps, woff2T_s, hid_s, start=True, stop=True)
                    nc.scalar.activation(off_s[:, sl], off_ps, ACTF.Identity, bias=boff2_s)
                nc.sync.dma_start(out=offd[b, :, :], in_=off_s)
                offx = work.tile([128, O * PT], f32, tag='offx')
                offy = work.tile([128, O * PT], f32, tag='offy')
                nc.vector.memset(offx, 0.0)
                nc.vector.memset(offy, 0.0)
                for xy, dst in ((0, offx), (1, offy)):
                    for o in range(O):
                        base = (b * 18 + xy * O + o) * HW
                        nc.sync.dma_start(
                            out=dst[:, o * PT:o * PT + 24],
                            in_=dap(offd, base, [[1, 128], [128, 24]]))
                        nc.sync.dma_start(
                            out=dst[:64, o * PT + 24:o * PT + 25],
                            in_=dap(offd, base + 24 * 128, [[1, 64], [128, 1]]))

                # ---- coords / weights / indices ----
                px = work.tile([128, O * PT], f32, tag='px')
                py = work.tile([128, O * PT], f32, tag='py')
                nc.vector.tensor_scalar(px, offx, float(OS * 0.5 * (W - 1)), None, ALU.mult)
                nc.vector.tensor_tensor(px, px, xgc2_s, ALU.add)
                nc.vector.tensor_scalar(px, px, 0.0, float(W - 1), ALU.max, ALU.min)
                nc.vector.tensor_scalar(py, offy, float(OS * 0.5 * (H - 1)), None, ALU.mult)
                nc.vector.tensor_tensor(py, py, ygc2_s, ALU.add)
                nc.vector.tensor_scalar(py, py, 0.0, float(H - 1), ALU.max, ALU.min)
                MAGIC = 12582912.0  # 1.5 * 2**23: float32 round-to-int trick
                x0f = work.tile([128, O * PT], f32, tag='x0f')
                y0f = work.tile([128, O * PT], f32, tag='y0f')
                nc.vector.tensor_scalar(x0f, px, -0.5, MAGIC, ALU.add, ALU.add)
                nc.vector.tensor_scalar(x0f, x0f, -MAGIC, None, ALU.add)
                nc.vector.tensor_scalar(y0f, py, -0.5, MAGIC, ALU.add, ALU.add)
                nc.vector.tensor_scalar(y0f, y0f, -MAGIC, None, ALU.add)
                wx = work.tile([128, O * PT], f32, tag='wx')
                wy = work.tile([128, O * PT], f32, tag='wy')
                nc.vector.tensor_tensor(wx, px, x0f, ALU.subtract)
                nc.vector.tensor_tensor(wy, py, y0f, ALU.subtract)
                r0 = work.tile([128, O * PT], f32, tag='r0')
                nc.vector.tensor_scalar(r0, y0f, float(W), None, ALU.mult)
                nc.vector.tensor_tensor(r0, r0, x0f, ALU.add)
                nc.vector.tensor_scalar(r0, r0, float(O), None, ALU.mult)
                nc.vector.tensor_tensor(r0, r0, obase2_s, ALU.add)
                nc.vector.tensor_scalar(r0, r0, 0.0, float(NP * O - 1), ALU.max, ALU.min)
                idx16 = work.tile([128, O * PT], i16, tag='idx')
                nc.vector.tensor_copy(idx16, r0)
                nc.sync.dma_start(
                    out=dap(idxd, b * O * NP, [[1, 128], [NP, O], [128, PT]]),
                    in_=idx16)
                idxw = big.tile([128, O * (NP // 16)], i16, tag='idxw')
                for rep in range(8):
                    nc.sync.dma_start(
                        out=idxw[rep * 16:(rep + 1) * 16, :],
                        in_=dap(idxd, b * O * NP,
                                [[1, 16], [NP, O], [16, NP // 16]]))
                w4 = big.tile([128, O * PT, 4], b16d, tag='w4')
                onemwx = work.tile([128, O * PT], f32, tag='o1x')
                onemwy = work.tile([128, O * PT], f32, tag='o1y')
                nc.vector.tensor_scalar(onemwx, wx, -1.0, 1.0, ALU.mult, ALU.add)
                nc.vector.tensor_scalar(onemwy, wy, -1.0, 1.0, ALU.mult, ALU.add)
                wtmp = work.tile([128, O * PT], f32, tag='wtmp')
                for n, (aa, cc2) in enumerate(((onemwx, onemwy), (wx, onemwy),
                                               (onemwx, wy), (wx, wy))):
                    nc.vector.tensor_tensor(wtmp, aa, cc2, ALU.mult)
                    nc.vector.tensor_copy(w4[:, :, n], wtmp)

                # ---- P projection ----
                if STAGE < 2:
                    continue
                p_sb = big.tile([128, 26, O * 64], b16d, tag='p_sb')
                nc.vector.memset(p_sb[:, 24:26, :], 0.0)
                for it in range(PT):
                    rows = 128 if it < 24 else HW - 24 * 128
                    for n2 in range(2):
                        pps = misc_ps.tile([128, 288], f32, tag='mps')
                        for cc in range(2):
                            nc.tensor.matmul(
                                pps[:rows],
                                kv_bf[:, cc, it * 128:it * 128 + rows],
                                wpack_s[:, cc, n2 * 288:(n2 + 1) * 288],
                                start=(cc == 0), stop=(cc == 1))
                        nc.scalar.activation(p_sb[:rows, it, n2 * 288:(n2 + 1) * 288],
                                             pps[:rows], ACTF.Copy)
                # ---- P4 build ----
                for n, dlt in enumerate((0, 1, 56, 57)):
                    lo = dlt
                    first = 128 - lo
                    for o in range(O):
                        base = b * NP * O * 256 + o * 256 + n * 64
                        nc.sync.dma_start(
                            out=dap(p4d, base,
                                    [[O * 256, first], [128 * O * 256, PT], [1, 64]]),
                            in_=p_sb[lo:128, 0:PT, o * 64:(o + 1) * 64])
                        if lo > 0:
                            nc.sync.dma_start(
                                out=dap(p4d, base + first * O * 256,
                                        [[O * 256, lo], [128 * O * 256, PT], [1, 64]]),
                                in_=p_sb[0:lo, 1:PT + 1, o * 64:(o + 1) * 64])

                # ---- gather + bilinear ----
                if STAGE < 3:
                    continue
                for o in range(O):
                    gt = gpool.tile([128, PT, 256], b16d, tag='gt', bufs=2)
                    nc.gpsimd.dma_gather(
                        out_ap=gt,
                        in_ap=dap(p4d, b * NP * O * 256, [[256, NP * O], [1, 256]]),
                        idxs_ap=idxw[:, o * (NP // 16):(o + 1) * (NP // 16)],
                        num_idxs=NP, num_idxs_reg=NP,
                        elem_size=256, elem_step=256, single_packet=False)
                    if SUB < 1:
                        continue
                    w4x = gpool.tile([128, PT, 4, 64], b16d, tag='w4x')
                    w4sl = w4[:, o * PT:(o + 1) * PT, :]
                    nc.gpsimd.tensor_copy(
                        w4x,
                        bass.AP(tensor=w4sl.tensor, offset=w4sl.offset,
                                ap=[*w4sl.ap, [0, 64]]))
                    if SUB < 2:
                        continue
                    nc.vector.tensor_tensor(
                        gt.rearrange('l t c -> l (t c)'),
                        gt.rearrange('l t c -> l (t c)'),
                        w4x.rearrange('l t n c -> l (t n c)'), ALU.mult)
                    gt4 = gt.rearrange('l t (n c) -> l t n c', c=64)
                    t12 = gpool.tile([128, PT, 2, 64], b16d, tag='t12')
                    nc.vector.tensor_tensor(t12, gt4[:, :, 0:2, :], gt4[:, :, 2:4, :],
                                            ALU.add)
                    if o == 0:
                        nc.vector.tensor_tensor(kvt_all[b][:, :, 0:64],
                                                t12[:, :, 0, :], t12[:, :, 1, :],
                                                ALU.add)
                    else:
                        nc.vector.tensor_tensor(kvt_all[b][:, :, 0:64],
                                                kvt_all[b][:, :, 0:64], t12[:, :, 0, :],
                                                ALU.add)
                        nc.vector.tensor_tensor(kvt_all[b][:, :, 0:64],
                                                kvt_all[b][:, :, 0:64], t12[:, :, 1, :],
                                                ALU.add)

                # ---- k transposes ----
                for t in (range(PT) if SUB >= 3 else []):
                    tp_ps = misc_ps.tile([HD, 128], b16d, tag='mps')
                    nc.tensor.transpose(tp_ps, kvt_all[b][:, t, 0:HD], ident)
                    nc.scalar.activation(k4_all[b][:, t * 128:(t + 1) * 128], tp_ps,
                                         ACTF.Copy)

                # ---- q_h scaled ----
                for ch in (range(NCH) if SUB >= 4 else []):
                    sl = slice(ch * IC, (ch + 1) * IC)
                    q_ps = misc_ps.tile([HD, IC], f32, tag='mps')
                    for cc in range(2):
                        nc.tensor.matmul(q_ps, wqhT_s[:, cc, :], x_bf[:, cc, sl],
                                         start=(cc == 0), stop=(cc == 1))
                    nc.scalar.activation(qs_all[b][:, sl], q_ps, ACTF.Copy,
                                         scale=float(SCALE))

            # ---------- attention ----------
            if STAGE < 4:
                for b in range(B):
                    dummy = work.tile([128, HW], b16d, tag='dmy')
                    nc.vector.memset(dummy, 0.25)
                    for cc in range(2):
                        nc.sync.dma_start(out=outp[b, cc * 128:(cc + 1) * 128, :],
                                          in_=dummy)
            jgroups = [(j, min(j + 2, PT)) for j in range(0, PT, 2)]
            for ic in (range(NCH) if STAGE >= 4 else []):
                yi0 = ic * 8
                avs = [av_ps.tile([33, IC], f32, tag=f'av{b}', bufs=1,
                                  name=f'av_{ic}_{b}') for b in range(B)]
                for (ja, jb) in jgroups:
                    jn = jb - ja
                    eb = ntt.tile([128, 2, IC], b16d, tag='eb')
                    for r in range(jn):
                        jt = ja + r
                        j0 = jt * 128
                        jend = min(j0 + 128, HW)
                        seg = j0
                        while seg < jend:
                            y_j = seg // W
                            seg_end = min((y_j + 1) * W, jend)
                            cnt = seg_end - seg
                            x_j0 = seg - y_j * W
                            basee = (x_j0 * 111 * W
                                     + (yi0 - y_j + (H - 1)) * W)
                            nc.sync.dma_start(
                                out=eb[seg - j0:seg - j0 + cnt, r, :]
                                    .rearrange('l (a c) -> l a c', a=8),
                                in_=dap(ebtab, basee,
                                        [[111 * W, cnt], [W, 8], [1, W]]))
                            seg = seg_end
                        if jend < j0 + 128:
                            nc.vector.memset(eb[jend - j0:128, r, :], 0.0)
                    for b in range(B):
                        stp = st_ps.tile([128, 2, 512], f32, tag='stp')
                        for r in range(jn):
                            jt = ja + r
                            nc.tensor.matmul(
                                stp[:, r, 0:IC],
                                k4_all[b][:, jt * 128:(jt + 1) * 128],
                                qs_all[b][:, ic * IC:(ic + 1) * IC],
                                start=True, stop=True)
                        nt = ntt.tile([128, 2, 512], b16d, tag='nt')
                        nc.scalar.activation(
                            nt[:, 0:jn, :].rearrange('l a c -> l (a c)'),
                            stp[:, 0:jn, :].rearrange('l a c -> l (a c)'),
                            ACTF.Exp)
                        nc.vector.tensor_tensor(
                            nt[:, 0:jn, 0:IC], nt[:, 0:jn, 0:IC],
                            eb[:, 0:jn, :], ALU.mult)
                        for r in range(jn):
                            jt = ja + r
                            nc.tensor.matmul(
                                avs[b], kvt_all[b][:, jt, 32:65], nt[:, r, 0:IC],
                                start=(jt == 0), stop=(jt == PT - 1),
                                skip_group_check=True)
                for b in range(B):
                    o1 = work.tile([33, IC], f32, tag='o1')
                    nc.scalar.activation(o1, avs[b], ACTF.Copy)
                    rc = work.tile([1, IC], f32, tag='rc')
                    nc.vector.reciprocal(rc, o1[32:33, :])
                    rb_ps = misc_ps.tile([HD, IC], f32, tag='mps')
                    nc.tensor.matmul(rb_ps, ones32, rc, start=True, stop=True)
                    att = work.tile([HD, IC], b16d, tag='att')
                    nc.vector.tensor_tensor(att, o1[0:HD, :], rb_ps, ALU.mult)
                    for mc in range(2):
                        wo_ps = misc_ps.tile([128, IC], f32, tag='mps')
                        nc.tensor.matmul(wo_ps, wouthT_s[:, mc * 128:(mc + 1) * 128],
                                         att, start=True, stop=True)
                        osb = work.tile([128, IC], b16d, tag='osb')
                        nc.scalar.activation(osb, wo_ps, ACTF.Copy)
                        nc.sync.dma_start(
                            out=outp[b, mc * 128:(mc + 1) * 128, ic * IC:(ic + 1) * IC],
                            in_=osb)
    nc.finalize()
    return nc


def _get_state():
    if 'nc' not in _STATE:
        _STATE['nc'] = _build_program()
    return _STATE['nc']


def _make_runner(nc):
    """Build (once) a cached jitted shard_map executable for the bass module.

    Mirrors concourse.bass2jax.run_bass_via_pjrt's multi-core path, but
    keeps the jitted callable across kernel() calls so only data transfer
    and execution happen per call.
    """
    import jax
    import numpy as _np
    from jax.sharding import Mesh, PartitionSpec
    from jax.experimental.shard_map import shard_map
    import concourse.mybir as mybir
    from concourse.bass2jax import (_bass_exec_p, install_neuronx_cc_hook,
                                    partition_id_tensor)

    install_neuronx_cc_hook()
    partition_name = nc.partition_id_tensor.name if nc.partition_id_tensor else None
    in_names, out_names, out_avals, zero_shapes = [], [], [], []
    for alloc in nc.m.functions[0].allocations:
        if not isinstance(alloc, mybir.MemoryLocationSet):
            continue
        name = alloc.memorylocations[0].name
        if alloc.kind == 'ExternalInput':
            if name != partition_name:
                in_names.append(name)
        elif alloc.kind == 'ExternalOutput':
            out_names.append(name)
            shape = tuple(alloc.tensor_shape)
            dtype = mybir.dt.np(alloc.dtype)
            out_avals.append(jax.core.ShapedArray(shape, dtype))
            zero_shapes.append((shape, dtype))
    n_params = len(in_names)
    n_outs = len(out_avals)
    all_in_names = list(in_names) + list(out_names)
    if partition_name is not None:
        all_in_names.append(partition_name)
    donate = tuple(range(n_params, n_params + n_outs))

    def _body(*args):
        operands = list(args)
        if partition_name is not None:
            operands.append(partition_id_tensor())
        return tuple(_bass_exec_p.bind(
            *operands, out_avals=tuple(out_avals), in_names=tuple(all_in_names),
            out_names=tuple(out_names), lowering_input_output_aliases=(),
            sim_require_finite=True, sim_require_nnan=True, nc=nc))

    devices = jax.devices()[:HEADS]
    mesh = Mesh(_np.asarray(devices), ('core',))
    shared = {'x32', 'kv32', 'wfoldT', 'boff1c', 'woff2T', 'boff2c',
              'xgc2', 'ygc2', 'obase2'}
    in_specs = tuple(PartitionSpec() if nm in shared else PartitionSpec('core')
                     for nm in in_names) + (PartitionSpec('core'),) * n_outs
    out_specs = (PartitionSpec('core'),) * n_outs
    sharded = jax.jit(
        shard_map(_body, mesh=mesh, in_specs=in_specs, out_specs=out_specs,
                  check_rep=False),
        keep_unused=True)

    # zero output-backing buffers: uploaded to the mesh ONCE and reused
    # (no donation, so they are never consumed)
    from jax.sharding import NamedSharding
    zs = NamedSharding(mesh, PartitionSpec('core'))
    dev_zeros = [jax.device_put(
        _np.zeros((HEADS * sh[0], *sh[1:]), dt), zs) for sh, dt in zero_shapes]

    def run(in_maps):
        concat_in = [
            _np.asarray(in_maps[0][nm]) if nm in shared else
            _np.concatenate([_np.asarray(in_maps[c][nm]) for c in range(HEADS)], axis=0)
            for nm in in_names]
        out_arrs = sharded(*concat_in, *dev_zeros)
        return {nm: _np.asarray(out_arrs[i]) for i, nm in enumerate(out_names)}

    return run


def kernel(query_map, kv_map, Wq, Wk, Wv, Woff1, boff1, Woff2, boff2,
           cpb_w1, cpb_b1, cpb_w2, cpb_b2, Wout, bout):
    inputs = dict(query_map=query_map, kv_map=kv_map, Wq=Wq, Wk=Wk, Wv=Wv,
                  Woff1=Woff1, boff1=boff1, Woff2=Woff2, boff2=boff2,
                  cpb_w1=cpb_w1, cpb_b1=cpb_b1, cpb_w2=cpb_w2, cpb_b2=cpb_b2,
                  Wout=Wout, bout=bout)
    nc = _get_state()
    if 'runner' not in _STATE:
        _STATE['runner'] = _make_runner(nc)
    in_maps = _host_prep(inputs)
    outs = _STATE['runner'](in_maps)
    part = outs['outp'].reshape(HEADS, B, DIM, HW).astype(np.float32)
    out = part.sum(axis=0)
    out += np.asarray(bout, np.float32)[None, :, None]
    return out.reshape(B, DIM, H, W)


# revision 27
# speedup vs baseline: 1.9370x; 1.0381x over previous
"""Deformable cross-attention on 8 trn2 NeuronCores via Bass/Tile.

Sharding: core c owns head c (both batch elements); host sums the 8
per-head partials of the output projection.

Per core: offset MLP (Woff1@Wq folded on host), per-head per-offset
projected kv maps P via PE matmuls (bf16), one-descriptor-per-sample
bilinear gather from a quad-row P4 layout via dma_gather, flash-style
attention over key tiles with multiplicative exp(bias) windows, and the
per-head slice of the output projection.
"""

import numpy as np
import ml_dtypes

bf16 = ml_dtypes.bfloat16

B, DIM, H, W = 2, 256, 56, 56
HEADS, O, OS = 8, 9, 0.1
HD = DIM // HEADS          # 32
HW = H * W                 # 3136
SCALE = np.float32(HD ** -0.5)
PT = 25                    # position tiles of 128
NP = PT * 128              # 3200 padded positions
IC = 448                   # attention i-chunk (8 rows of y_i)
NCH = HW // IC             # 7
EBT = 111 * 111

_STATE: dict = {}


def _host_prep(inputs):
    f32 = np.float32
    Wq = np.asarray(inputs['Wq'], f32)
    Wk = np.asarray(inputs['Wk'], f32)
    Wv = np.asarray(inputs['Wv'], f32)
    Woff1 = np.asarray(inputs['Woff1'], f32)
    Woff2 = np.asarray(inputs['Woff2'], f32)
    Wout = np.asarray(inputs['Wout'], f32)
    wfoldT = np.ascontiguousarray((Woff1 @ Wq).T).astype(bf16)       # (256,64)
    woff2T = np.ascontiguousarray(Woff2.T).astype(bf16)              # (64,18)
    boff1c = np.asarray(inputs['boff1'], f32).reshape(64, 1)
    boff2c = np.asarray(inputs['boff2'], f32).reshape(18, 1)

    dyv = np.arange(-(H - 1), H, dtype=f32) / (H - 1)
    dxv = np.arange(-(W - 1), W, dtype=f32) / (W - 1)
    gy, gx = np.meshgrid(dyv, dxv, indexing='ij')
    coords = np.stack([gy, gx], -1).reshape(-1, 2).astype(f32)
    hdn = np.maximum(coords @ np.asarray(inputs['cpb_w1'], f32).T
                     + np.asarray(inputs['cpb_b1'], f32), 0.0)
    table = hdn @ np.asarray(inputs['cpb_w2'], f32).T + np.asarray(inputs['cpb_b2'], f32)
    ebt = np.exp(table.astype(f32))                                  # (12321, 8)
    # expand over x_j so EB window DMAs have all-positive steps:
    # ebx[h, x_j, a, x_i] = exp(T2)[a, x_i + 55 - x_j, h]
    eb2 = ebt.reshape(111, 111, HEADS)
    xj = np.arange(W)[:, None, None]
    aa = np.arange(111)[None, :, None]
    xi = np.arange(W)[None, None, :]
    ebx = eb2[aa, xi + (W - 1) - xj, :].astype(bf16)      # (56, 111, 56, 8)

    pos = np.arange(NP)
    py_pos = np.where(pos < HW, pos // W, 0)
    px_pos = np.where(pos < HW, pos % W, 0)
    gxl = np.linspace(-1.0, 1.0, W, dtype=f32)
    gyl = np.linspace(-1.0, 1.0, H, dtype=f32)
    basex = ((gxl[px_pos] + 1.0) * 0.5 * (W - 1)).astype(f32)
    basey = ((gyl[py_pos] + 1.0) * 0.5 * (H - 1)).astype(f32)
    lane = pos % 128
    tcol = pos // 128
    xgc2 = np.zeros((128, O * PT), f32)
    ygc2 = np.zeros((128, O * PT), f32)
    obase2 = np.zeros((128, O * PT), f32)
    for o in range(O):
        xgc2[lane, o * PT + tcol] = basex
        ygc2[lane, o * PT + tcol] = basey
        obase2[lane, o * PT + tcol] = o

    x32 = np.ascontiguousarray(
        np.asarray(inputs['query_map'], f32).reshape(B, DIM, HW)).astype(bf16)
    kv32 = np.ascontiguousarray(
        np.asarray(inputs['kv_map'], f32).reshape(B, DIM, HW)).astype(bf16)
    per_core = []
    for h in range(HEADS):
        wpack = np.zeros((DIM, O * 64), f32)
        for o in range(O):
            wpack[:, o * 64:o * 64 + 32] = Wk[h * HD:(h + 1) * HD, o * DIM:(o + 1) * DIM].T
            wpack[:, o * 64 + 32:o * 64 + 64] = Wv[h * HD:(h + 1) * HD, o * DIM:(o + 1) * DIM].T
        blob16 = np.concatenate([
            np.ascontiguousarray(Wq[h * HD:(h + 1) * HD].T).astype(bf16).reshape(-1),
            wpack.astype(bf16).reshape(-1),
            np.ascontiguousarray(Wout[:, h * HD:(h + 1) * HD].T).astype(bf16).reshape(-1),
            np.ascontiguousarray(ebx[..., h]).reshape(-1),
            wfoldT.reshape(-1), woff2T.reshape(-1)]).reshape(1, -1)
        blob32 = np.concatenate([
            boff1c.reshape(-1), boff2c.reshape(-1),
            xgc2.reshape(-1), ygc2.reshape(-1), obase2.reshape(-1)]).reshape(1, -1)
        per_core.append({'x32': x32, 'kv32': kv32,
                         'blob16': blob16, 'blob32': blob32})
    return per_core


def _build_program():
    import os
    STAGE = int(os.environ.get('KSTAGE', '4'))
    SUB = int(os.environ.get('KSUB', '9'))
    import concourse.bass as bass
    import concourse.bacc as baccm
    import concourse.tile as tile
    from concourse import mybir
    from concourse.masks import make_identity
    import contextlib

    f32 = mybir.dt.float32
    b16d = mybir.dt.bfloat16
    i16 = mybir.dt.int16
    ALU = mybir.AluOpType
    ACTF = mybir.ActivationFunctionType

    nc = baccm.Bacc(trn_type="TRN2")

    x32 = nc.dram_tensor('x32', [B, DIM, HW], b16d, kind='ExternalInput')
    kv32 = nc.dram_tensor('kv32', [B, DIM, HW], b16d, kind='ExternalInput')
    NB16 = DIM * HD + DIM * O * 64 + HD * DIM + W * 111 * W + DIM * 64 + 64 * 18
    NB32 = 64 + 18 + 3 * 128 * O * PT
    blob16 = nc.dram_tensor('blob16', [1, NB16], b16d, kind='ExternalInput')
    blob32 = nc.dram_tensor('blob32', [1, NB32], f32, kind='ExternalInput')
    OFF_WQHT = 0
    OFF_WPACK = OFF_WQHT + DIM * HD
    OFF_WOUTHT = OFF_WPACK + DIM * O * 64
    OFF_EBTAB = OFF_WOUTHT + HD * DIM
    OFF_WFOLDT = OFF_EBTAB + W * 111 * W
    OFF_WOFF2T = OFF_WFOLDT + DIM * 64
    OFF_B1 = 0
    OFF_B2 = OFF_B1 + 64
    OFF_XGC = OFF_B2 + 18
    OFF_YGC = OFF_XGC + 128 * O * PT
    OFF_OB = OFF_YGC + 128 * O * PT
    outp = nc.dram_tensor('outp', [B, DIM, HW], b16d, kind='ExternalOutput')

    offd = nc.dram_tensor('offd', [B, 18, HW], f32, kind='Internal')
    idxd = nc.dram_tensor('idxd', [B, O * NP], i16, kind='Internal')
    p4d = nc.dram_tensor('p4d', [B, NP * O * 256], b16d, kind='Internal')

    def dap(handle, off, dims):
        return bass.AP(tensor=handle, offset=off, ap=[list(d) for d in dims])

    with tile.TileContext(nc) as tc:
        ctx = contextlib.ExitStack()
        consts = ctx.enter_context(tc.tile_pool(name='consts', bufs=1))
        big = ctx.enter_context(tc.tile_pool(name='big', bufs=1))
        work = ctx.enter_context(tc.tile_pool(name='work', bufs=2))
        gpool = ctx.enter_context(tc.tile_pool(name='gpool', bufs=1))
        ntt = ctx.enter_context(tc.tile_pool(name='ntt', bufs=3))
        st_ps = ctx.enter_context(tc.tile_pool(name='st_ps', bufs=2, space='PSUM'))
        av_ps = ctx.enter_context(tc.tile_pool(name='av_ps', bufs=2, space='PSUM'))
        misc_ps = ctx.enter_context(tc.tile_pool(name='misc_ps', bufs=2, space='PSUM'))

        with ctx:
            wfoldT_s = consts.tile([128, 2, 64], b16d)
            nc.sync.dma_start(out=wfoldT_s, in_=dap(
                blob16, OFF_WFOLDT, [[64, 128], [128 * 64, 2], [1, 64]]))
            boff1_s = consts.tile([64, 1], f32)
            nc.sync.dma_start(out=boff1_s, in_=dap(blob32, OFF_B1, [[1, 64], [1, 1]]))
            woff2T_s = consts.tile([64, 18], b16d)
            nc.sync.dma_start(out=woff2T_s, in_=dap(
                blob16, OFF_WOFF2T, [[18, 64], [1, 18]]))
            boff2_s = consts.tile([18, 1], f32)
            nc.sync.dma_start(out=boff2_s, in_=dap(blob32, OFF_B2, [[1, 18], [1, 1]]))
            wqhT_s = consts.tile([128, 2, HD], b16d)
            nc.sync.dma_start(out=wqhT_s, in_=dap(
                blob16, OFF_WQHT, [[HD, 128], [128 * HD, 2], [1, HD]]))
            wpack_s = consts.tile([128, 2, O * 64], b16d)
            nc.sync.dma_start(out=wpack_s, in_=dap(
                blob16, OFF_WPACK, [[O * 64, 128], [128 * O * 64, 2], [1, O * 64]]))
            wouthT_s = consts.tile([HD, DIM], b16d)
            nc.sync.dma_start(out=wouthT_s, in_=dap(
                blob16, OFF_WOUTHT, [[DIM, HD], [1, DIM]]))
            xgc2_s = consts.tile([128, O * PT], f32)
            nc.sync.dma_start(out=xgc2_s, in_=dap(
                blob32, OFF_XGC, [[O * PT, 128], [1, O * PT]]))
            ygc2_s = consts.tile([128, O * PT], f32)
            nc.sync.dma_start(out=ygc2_s, in_=dap(
                blob32, OFF_YGC, [[O * PT, 128], [1, O * PT]]))
            obase2_s = consts.tile([128, O * PT], f32)
            nc.sync.dma_start(out=obase2_s, in_=dap(
                blob32, OFF_OB, [[O * PT, 128], [1, O * PT]]))
            ident = consts.tile([128, 128], b16d)
            make_identity(nc, ident)
            ones32 = consts.tile([1, HD], f32)
            nc.vector.memset(ones32, 1.0)

            kvt_all = [big.tile([128, PT, 65], b16d, tag=f'kvt{b}', name=f'kvt{b}') for b in range(B)]
            k4_all = [big.tile([HD, PT * 128], b16d, tag=f'k4{b}', name=f'k4{b}') for b in range(B)]
            qs_all = [big.tile([HD, HW], b16d, tag=f'qs{b}', name=f'qs{b}') for b in range(B)]
            for b in range(B):
                nc.vector.memset(kvt_all[b][:, :, 64:65], 1.0)

            for b in range(B):
                x_bf = big.tile([128, 2, HW], b16d, tag='x_bf')
                kv_bf = big.tile([128, 2, HW], b16d, tag='kv_bf')
                for cc in range(2):
                    nc.sync.dma_start(out=x_bf[:, cc, :],
                                      in_=x32[b, cc * 128:(cc + 1) * 128, :])
                    nc.sync.dma_start(out=kv_bf[:, cc, :],
                                      in_=kv32[b, cc * 128:(cc + 1) * 128, :])

                # ---- offset MLP ----
                off_s = big.tile([18, HW], f32, tag='off')
                for ch in range(NCH):
                    sl = slice(ch * IC, (ch + 1) * IC)
                    hid_ps = misc_ps.tile([64, IC], f32, tag='mps')
                    for cc in range(2):
                        nc.tensor.matmul(hid_ps, wfoldT_s[:, cc, :], x_bf[:, cc, sl],
                                         start=(cc == 0), stop=(cc == 1))
                    hid_s = work.tile([64, IC], b16d, tag='hid')
                    nc.scalar.activation(hid_s, hid_ps, ACTF.Gelu_apprx_tanh, bias=boff1_s)
                    off_ps = misc_ps.tile([18, IC], f32, tag='mps')
                    nc.tensor.matmul(off_ps, woff2T_s, hid_s, start=True, stop=True)
                    nc.scalar.activation(off_s[:, sl], off_ps, ACTF.Identity, bias=boff2_s)
                nc.sync.dma_start(out=offd[b, :, :], in_=off_s)
                offx = work.tile([128, O * PT], f32, tag='offx')
                offy = work.tile([128, O * PT], f32, tag='offy')
                nc.vector.memset(offx, 0.0)
                nc.vector.memset(offy, 0.0)
                for xy, dst in ((0, offx), (1, offy)):
                    for o in range(O):
                        base = (b * 18 + xy * O + o) * HW
                        nc.sync.dma_start(
                            out=dst[:, o * PT:o * PT + 24],
                            in_=dap(offd, base, [[1, 128], [128, 24]]))
                        nc.sync.dma_start(
                            out=dst[:64, o * PT + 24:o * PT + 25],
                            in_=dap(offd, base + 24 * 128, [[1, 64], [128, 1]]))

                # ---- coords / weights / indices ----
                px = work.tile([128, O * PT], f32, tag='px')
                py = work.tile([128, O * PT], f32, tag='py')
                nc.vector.tensor_scalar(px, offx, float(OS * 0.5 * (W - 1)), None, ALU.mult)
                nc.vector.tensor_tensor(px, px, xgc2_s, ALU.add)
                nc.vector.tensor_scalar(px, px, 0.0, float(W - 1), ALU.max, ALU.min)
                nc.vector.tensor_scalar(py, offy, float(OS * 0.5 * (H - 1)), None, ALU.mult)
                nc.vector.tensor_tensor(py, py, ygc2_s, ALU.add)
                nc.vector.tensor_scalar(py, py, 0.0, float(H - 1), ALU.max, ALU.min)
                MAGIC = 12582912.0  # 1.5 * 2**23: float32 round-to-int trick
                x0f = work.tile([128, O * PT], f32, tag='x0f')
                y0f = work.tile([128, O * PT], f32, tag='y0f')
                nc.vector.tensor_scalar(x0f, px, -0.5, MAGIC, ALU.add, ALU.add)
                nc.vector.tensor_scalar(x0f, x0f, -MAGIC, None, ALU.add)
                nc.vector.tensor_scalar(y0f, py, -0.5, MAGIC, ALU.add, ALU.add)
                nc.vector.tensor_scalar(y0f, y0f, -MAGIC, None, ALU.add)
                wx = work.tile([128, O * PT], f32, tag='wx')
                wy = work.tile([128, O * PT], f32, tag='wy')
                nc.vector.tensor_tensor(wx, px, x0f, ALU.subtract)
                nc.vector.tensor_tensor(wy, py, y0f, ALU.subtract)
                r0 = work.tile([128, O * PT], f32, tag='r0')
                nc.vector.tensor_scalar(r0, y0f, float(W), None, ALU.mult)
                nc.vector.tensor_tensor(r0, r0, x0f, ALU.add)
                nc.vector.tensor_scalar(r0, r0, float(O), None, ALU.mult)
                nc.vector.tensor_tensor(r0, r0, obase2_s, ALU.add)
                nc.vector.tensor_scalar(r0, r0, 0.0, float(NP * O - 1), ALU.max, ALU.min)
                idx16 = work.tile([128, O * PT], i16, tag='idx')
                nc.vector.tensor_copy(idx16, r0)
                nc.sync.dma_start(
                    out=dap(idxd, b * O * NP, [[1, 128], [NP, O], [128, PT]]),
                    in_=idx16)
                idxw = big.tile([128, O * (NP // 16)], i16, tag='idxw')
                for rep in range(8):
                    nc.sync.dma_start(
                        out=idxw[rep * 16:(rep + 1) * 16, :],
                        in_=dap(idxd, b * O * NP,
                                [[1, 16], [NP, O], [16, NP // 16]]))
                w4 = big.tile([128, O * PT, 4], b16d, tag='w4')
                onemwx = work.tile([128, O * PT], f32, tag='o1x')
                onemwy = work.tile([128, O * PT], f32, tag='o1y')
                nc.vector.tensor_scalar(onemwx, wx, -1.0, 1.0, ALU.mult, ALU.add)
                nc.vector.tensor_scalar(onemwy, wy, -1.0, 1.0, ALU.mult, ALU.add)
                wtmp = work.tile([128, O * PT], f32, tag='wtmp')
                for n, (aa, cc2) in enumerate(((onemwx, onemwy), (wx, onemwy),
                                               (onemwx, wy), (wx, wy))):
                    nc.vector.tensor_tensor(wtmp, aa, cc2, ALU.mult)
                    nc.vector.tensor_copy(w4[:, :, n], wtmp)

                # ---- P projection ----
                if STAGE < 2:
                    continue
                p_sb = big.tile([128, 26, O * 64], b16d, tag='p_sb')
                nc.vector.memset(p_sb[:, 24:26, :], 0.0)
                for it in range(PT):
                    rows = 128 if it < 24 else HW - 24 * 128
                    for n2 in range(2):
                        pps = misc_ps.tile([128, 288], f32, tag='mps')
                        for cc in range(2):
                            nc.tensor.matmul(
                                pps[:rows],
                                kv_bf[:, cc, it * 128:it * 128 + rows],
                                wpack_s[:, cc, n2 * 288:(n2 + 1) * 288],
                                start=(cc == 0), stop=(cc == 1))
                        nc.scalar.activation(p_sb[:rows, it, n2 * 288:(n2 + 1) * 288],
                                             pps[:rows], ACTF.Copy)
                # ---- P4 build ----
                for n, dlt in enumerate((0, 1, 56, 57)):
                    lo = dlt
                    first = 128 - lo
                    for o in range(O):
                        base = b * NP * O * 256 + o * 256 + n * 64
                        nc.sync.dma_start(
                            out=dap(p4d, base,
                                    [[O * 256, first], [128 * O * 256, PT], [1, 64]]),
                            in_=p_sb[lo:128, 0:PT, o * 64:(o + 1) * 64])
                        if lo > 0:
                            nc.sync.dma_start(
                                out=dap(p4d, base + first * O * 256,
                                        [[O * 256, lo], [128 * O * 256, PT], [1, 64]]),
                                in_=p_sb[0:lo, 1:PT + 1, o * 64:(o + 1) * 64])

                # ---- gather + bilinear ----
                if STAGE < 3:
                    continue
                for o in range(O):
                    gt = gpool.tile([128, PT, 256], b16d, tag='gt', bufs=2)
                    nc.gpsimd.dma_gather(
                        out_ap=gt,
                        in_ap=dap(p4d, b * NP * O * 256, [[256, NP * O], [1, 256]]),
                        idxs_ap=idxw[:, o * (NP // 16):(o + 1) * (NP // 16)],
                        num_idxs=NP, num_idxs_reg=NP,
                        elem_size=256, elem_step=256, single_packet=False)
                    if SUB < 1:
                        continue
                    w4x = gpool.tile([128, PT, 4, 64], b16d, tag='w4x')
                    w4sl = w4[:, o * PT:(o + 1) * PT, :]
                    nc.gpsimd.tensor_copy(
                        w4x,
                        bass.AP(tensor=w4sl.tensor, offset=w4sl.offset,
                                ap=[*w4sl.ap, [0, 64]]))
                    if SUB < 2:
                        continue
                    nc.vector.tensor_tensor(
                        gt.rearrange('l t c -> l (t c)'),
                        gt.rearrange('l t c -> l (t c)'),
                        w4x.rearrange('l t n c -> l (t n c)'), ALU.mult)
                    gt4 = gt.rearrange('l t (n c) -> l t n c', c=64)
                    t12 = gpool.tile([128, PT, 2, 64], b16d, tag='t12')
                    nc.vector.tensor_tensor(t12, gt4[:, :, 0:2, :], gt4[:, :, 2:4, :],
                                            ALU.add)
                    if o == 0:
                        nc.vector.tensor_tensor(kvt_all[b][:, :, 0:64],
                                                t12[:, :, 0, :], t12[:, :, 1, :],
                                                ALU.add)
                    else:
                        nc.vector.tensor_tensor(kvt_all[b][:, :, 0:64],
                                                kvt_all[b][:, :, 0:64], t12[:, :, 0, :],
                                                ALU.add)
                        nc.vector.tensor_tensor(kvt_all[b][:, :, 0:64],
                                                kvt_all[b][:, :, 0:64], t12[:, :, 1, :],
                                                ALU.add)

                # ---- k transposes ----
                for t in (range(PT) if SUB >= 3 else []):
                    tp_ps = misc_ps.tile([HD, 128], b16d, tag='mps')
                    nc.tensor.transpose(tp_ps, kvt_all[b][:, t, 0:HD], ident)
                    nc.scalar.activation(k4_all[b][:, t * 128:(t + 1) * 128], tp_ps,
                                         ACTF.Copy)

                # ---- q_h scaled ----
                for ch in (range(NCH) if SUB >= 4 else []):
                    sl = slice(ch * IC, (ch + 1) * IC)
                    q_ps = misc_ps.tile([HD, IC], f32, tag='mps')
                    for cc in range(2):
                        nc.tensor.matmul(q_ps, wqhT_s[:, cc, :], x_bf[:, cc, sl],
                                         start=(cc == 0), stop=(cc == 1))
                    nc.scalar.activation(qs_all[b][:, sl], q_ps, ACTF.Copy,
                                         scale=float(SCALE))

            # ---------- attention ----------
            if STAGE < 4:
                for b in range(B):
                    dummy = work.tile([128, HW], b16d, tag='dmy')
                    nc.vector.memset(dummy, 0.25)
                    for cc in range(2):
                        nc.sync.dma_start(out=outp[b, cc * 128:(cc + 1) * 128, :],
                                          in_=dummy)
            jgroups = [(j, min(j + 2, PT)) for j in range(0, PT, 2)]
            for ic in (range(NCH) if STAGE >= 4 else []):
                yi0 = ic * 8
                avs = [av_ps.tile([33, IC], f32, tag=f'av{b}', bufs=1,
                                  name=f'av_{ic}_{b}') for b in range(B)]
                for (ja, jb) in jgroups:
                    jn = jb - ja
                    eb = ntt.tile([128, 2, IC], b16d, tag='eb')
                    for r in range(jn):
                        jt = ja + r
                        j0 = jt * 128
                        jend = min(j0 + 128, HW)
                        seg = j0
                        while seg < jend:
                            y_j = seg // W
                            seg_end = min((y_j + 1) * W, jend)
                            cnt = seg_end - seg
                            x_j0 = seg - y_j * W
                            basee = (x_j0 * 111 * W
                                     + (yi0 - y_j + (H - 1)) * W)
                            nc.sync.dma_start(
                                out=eb[seg - j0:seg - j0 + cnt, r, :]
                                    .rearrange('l (a c) -> l a c', a=8),
                                in_=dap(blob16, OFF_EBTAB + basee,
                                        [[111 * W, cnt], [W, 8], [1, W]]))
                            seg = seg_end
                        if jend < j0 + 128:
                            nc.vector.memset(eb[jend - j0:128, r, :], 0.0)
                    for b in range(B):
                        stp = st_ps.tile([128, 2, 512], f32, tag='stp')
                        for r in range(jn):
                            jt = ja + r
                            nc.tensor.matmul(
                                stp[:, r, 0:IC],
                                k4_all[b][:, jt * 128:(jt + 1) * 128],
                                qs_all[b][:, ic * IC:(ic + 1) * IC],
                                start=True, stop=True)
                        nt = ntt.tile([128, 2, 512], b16d, tag='nt')
                        nc.scalar.activation(
                            nt[:, 0:jn, :].rearrange('l a c -> l (a c)'),
                            stp[:, 0:jn, :].rearrange('l a c -> l (a c)'),
                            ACTF.Exp)
                        nc.vector.tensor_tensor(
                            nt[:, 0:jn, 0:IC], nt[:, 0:jn, 0:IC],
                            eb[:, 0:jn, :], ALU.mult)
                        for r in range(jn):
                            jt = ja + r
                            nc.tensor.matmul(
                                avs[b], kvt_all[b][:, jt, 32:65], nt[:, r, 0:IC],
                                start=(jt == 0), stop=(jt == PT - 1),
                                skip_group_check=True)
                for b in range(B):
                    o1 = work.tile([33, IC], f32, tag='o1')
                    nc.scalar.activation(o1, avs[b], ACTF.Copy)
                    rc = work.tile([1, IC], f32, tag='rc')
                    nc.vector.reciprocal(rc, o1[32:33, :])
                    rb_ps = misc_ps.tile([HD, IC], f32, tag='mps')
                    nc.tensor.matmul(rb_ps, ones32, rc, start=True, stop=True)
                    att = work.tile([HD, IC], b16d, tag='att')
                    nc.vector.tensor_tensor(att, o1[0:HD, :], rb_ps, ALU.mult)
                    for mc in range(2):
                        wo_ps = misc_ps.tile([128, IC], f32, tag='mps')
                        nc.tensor.matmul(wo_ps, wouthT_s[:, mc * 128:(mc + 1) * 128],
                                         att, start=True, stop=True)
                        osb = work.tile([128, IC], b16d, tag='osb')
                        nc.scalar.activation(osb, wo_ps, ACTF.Copy)
                        nc.sync.dma_start(
                            out=outp[b, mc * 128:(mc + 1) * 128, ic * IC:(ic + 1) * IC],
                            in_=osb)
    nc.finalize()
    return nc


def _get_state():
    if 'nc' not in _STATE:
        _STATE['nc'] = _build_program()
    return _STATE['nc']


def _make_runner(nc):
    """Build (once) a cached jitted shard_map executable for the bass module.

    Mirrors concourse.bass2jax.run_bass_via_pjrt's multi-core path, but
    keeps the jitted callable across kernel() calls so only data transfer
    and execution happen per call.
    """
    import jax
    import numpy as _np
    from jax.sharding import Mesh, PartitionSpec
    from jax.experimental.shard_map import shard_map
    import concourse.mybir as mybir
    from concourse.bass2jax import (_bass_exec_p, install_neuronx_cc_hook,
                                    partition_id_tensor)

    install_neuronx_cc_hook()
    partition_name = nc.partition_id_tensor.name if nc.partition_id_tensor else None
    in_names, out_names, out_avals, zero_shapes = [], [], [], []
    for alloc in nc.m.functions[0].allocations:
        if not isinstance(alloc, mybir.MemoryLocationSet):
            continue
        name = alloc.memorylocations[0].name
        if alloc.kind == 'ExternalInput':
            if name != partition_name:
                in_names.append(name)
        elif alloc.kind == 'ExternalOutput':
            out_names.append(name)
            shape = tuple(alloc.tensor_shape)
            dtype = mybir.dt.np(alloc.dtype)
            out_avals.append(jax.core.ShapedArray(shape, dtype))
            zero_shapes.append((shape, dtype))
    n_params = len(in_names)
    n_outs = len(out_avals)
    all_in_names = list(in_names) + list(out_names)
    if partition_name is not None:
        all_in_names.append(partition_name)
    donate = tuple(range(n_params, n_params + n_outs))

    def _body(*args):
        operands = list(args)
        if partition_name is not None:
            operands.append(partition_id_tensor())
        return tuple(_bass_exec_p.bind(
            *operands, out_avals=tuple(out_avals), in_names=tuple(all_in_names),
            out_names=tuple(out_names), lowering_input_output_aliases=(),
            sim_require_finite=True, sim_require_nnan=True, nc=nc))

    devices = jax.devices()[:HEADS]
    mesh = Mesh(_np.asarray(devices), ('core',))
    shared = {'x32', 'kv32', 'blob32'}
    in_specs = tuple(PartitionSpec() if nm in shared else PartitionSpec('core')
                     for nm in in_names) + (PartitionSpec('core'),) * n_outs
    out_specs = (PartitionSpec('core'),) * n_outs
    sharded = jax.jit(
        shard_map(_body, mesh=mesh, in_specs=in_specs, out_specs=out_specs,
                  check_rep=False),
        keep_unused=True)

    # zero output-backing buffers: uploaded to the mesh ONCE and reused
    # (no donation, so they are never consumed)
    from jax.sharding import NamedSharding
    zs = NamedSharding(mesh, PartitionSpec('core'))
    dev_zeros = [jax.device_put(
        _np.zeros((HEADS * sh[0], *sh[1:]), dt), zs) for sh, dt in zero_shapes]

    def run(in_maps):
        concat_in = [
            _np.asarray(in_maps[0][nm]) if nm in shared else
            _np.concatenate([_np.asarray(in_maps[c][nm]) for c in range(HEADS)], axis=0)
            for nm in in_names]
        out_arrs = sharded(*concat_in, *dev_zeros)
        return {nm: _np.asarray(out_arrs[i]) for i, nm in enumerate(out_names)}

    return run


def kernel(query_map, kv_map, Wq, Wk, Wv, Woff1, boff1, Woff2, boff2,
           cpb_w1, cpb_b1, cpb_w2, cpb_b2, Wout, bout):
    inputs = dict(query_map=query_map, kv_map=kv_map, Wq=Wq, Wk=Wk, Wv=Wv,
                  Woff1=Woff1, boff1=boff1, Woff2=Woff2, boff2=boff2,
                  cpb_w1=cpb_w1, cpb_b1=cpb_b1, cpb_w2=cpb_w2, cpb_b2=cpb_b2,
                  Wout=Wout, bout=bout)
    nc = _get_state()
    if 'runner' not in _STATE:
        _STATE['runner'] = _make_runner(nc)
    in_maps = _host_prep(inputs)
    outs = _STATE['runner'](in_maps)
    part = outs['outp'].reshape(HEADS, B, DIM, HW).astype(np.float32)
    out = part.sum(axis=0)
    out += np.asarray(bout, np.float32)[None, :, None]
    return out.reshape(B, DIM, H, W)
